# revision 1
# baseline (speedup 1.0000x reference)
"""Trainium2 Bass kernel for the CGC (multi-task MoE) layer.

Reference computation (all-dense MoE, T=2 tasks, E=6 experts, EC=4 per task):
    h1 = relu(x @ We1[e] + be1[e])            [B, E, H1]
    h2 = relu(h1 @ We2[e] + be2[e])           [B, E, H2]
    g  = relu(x @ Wg1[t] + bg1[t])            [B, T, G]
    gate = softmax(g @ Wgs[t])                [B, T, EC]
    out[t, b, :] = sum_j gate[b, t, j] * h2[b, IDX[t, j], :]

Sharding: data-parallel over batch across 8 NeuronCores (B=8192 -> 1024
rows/core), weights replicated, no collectives.  The host pre-transposes each
x shard to xT[D, BC] so every matmul's contraction dim sits on SBUF
partitions with no on-device transposes:

    L1:  psum[h, b] += We1[d, h].T-block @ xT[d, b]      (lhsT=We1, rhs=xT)
    L2:  psum[b, o] += h1T[h, b].T-block @ We2[h, o]     (lhsT=h1T, rhs=We2)

Default operand dtype is bf16: same PE rate as f32r but half the DMA
traffic and SBUF footprint (rel err ~2e-3 vs the 2e-2 gate).  Fat matmul
chains run 2-wide with psum pools double-buffering groups; the tiny logit
chains all share one psum bank so they interleave 16-wide.  be2 is folded into the L2 PSUM accumulation with a K=1 ones-matmul.
relu+gate scaling is fused on ScalarE (gate > 0 so relu(g*x) = g*relu(x)).
"""

import numpy as np

import concourse.bass as bass
import concourse.mybir as mybir
import concourse.tile as tile
from concourse import bacc

# ---- problem shapes (hardcoded per contest contract) ----
B, D, H1, H2, G = 8192, 2048, 1024, 512, 512
T, SPEC, SH = 2, 2, 2
E = T * SPEC + SH          # 6
EC = SPEC + SH             # 4
NCORES = 8
BC = B // NCORES           # 1024 per-core batch
TASK_EXPERTS = [[0, 1, 4, 5], [2, 3, 4, 5]]   # IDX rows

F32 = mybir.dt.float32
F32R = mybir.dt.float32r
BF16 = mybir.dt.bfloat16
FP8 = mybir.dt.float8e4
P = 128
DRMODE = mybir.MatmulPerfMode.DoubleRow

# partial-fp8 config: first KF8 of 16 L1 d-tiles run as fp8 DoubleRow pairs
# (2.2x per-MAC), the rest bf16.  Scales: fp8 ops at x*4 / W*1024, bf16 ops
# at x*64 / W*64 so every product lands in PSUM at scale 4096, descaled in
# the relu activation.  KF8=6 -> rel err ~1.6e-2 (sim), vs the 2e-2 gate.
KF8_DEFAULT = 6
XS8, WS8, BS = 4.0, 1024.0, 64.0
PSC = 1.0 / (BS * BS)  # psum descale for the bf16-scaled accumulation

Relu = mybir.ActivationFunctionType.Relu
Exp = mybir.ActivationFunctionType.Exp
AxX = mybir.AxisListType.X


def build_program(bc=BC, bch=None, reps=1, mm_mode="bf16", psa=4,
                  psb=4, kf8=KF8_DEFAULT):
    """Build the SPMD single-core program.

    bc: per-core batch size; bch: L1/gate moving free-dim chunk; reps: body
    repetitions (for wall-clock differencing); mm_mode: f32 | f32r | bf16.
    """
    if bch is None:
        bch = min(512, bc)
    assert bc % P == 0 and bc % bch == 0
    KT, HT, GT = D // P, H1 // P, G // P
    NB, NBT = bc // bch, bc // P

    # matmul-operand dtype: walrus requires f32r matmul inputs to be produced
    # as f32r, so the whole producer chain (DRAM tensor -> DMA -> tile) is
    # declared f32r in that mode (same bytes as f32 host-side).
    io_dt = {"bf16": BF16, "f32r": F32R, "f32": F32}[mm_mode]
    if mm_mode != "bf16":
        kf8 = 0
    scaled = mm_mode == "bf16"   # x/We1/Wg1 arrive pre-scaled by 64
    psc = PSC if scaled else 1.0
    assert kf8 % 2 == 0

    nc = bacc.Bacc("TRN2", target_bir_lowering=False, debug=False,
                   num_devices=NCORES)

    xT = nc.dram_tensor("xT", [D, bc], io_dt, kind="ExternalInput")
    We1 = nc.dram_tensor("We1", [E, D, H1], io_dt, kind="ExternalInput")
    be1 = nc.dram_tensor("be1", [E, H1], F32, kind="ExternalInput")
    We2 = nc.dram_tensor("We2", [E, H1, H2], io_dt, kind="ExternalInput")
    be2 = nc.dram_tensor("be2", [E, H2], io_dt, kind="ExternalInput")
    Wg1 = nc.dram_tensor("Wg1", [T, D, G], io_dt, kind="ExternalInput")
    bg1 = nc.dram_tensor("bg1", [T, G], F32, kind="ExternalInput")
    Wgs = nc.dram_tensor("Wgs", [T, G, EC], io_dt, kind="ExternalInput")
    # K=1 all-ones lhsT used to broadcast be2 into the L2 PSUM accumulation;
    # an input tensor because only DMA can produce f32r-tagged data.
    ones_d = nc.dram_tensor("ones", [1, P], io_dt, kind="ExternalInput")
    if kf8:
        # pre-swizzled host layouts: [p, kp, two, .] flattened
        xT8 = nc.dram_tensor("xT8", [P, kf8 * bc], FP8,
                             kind="ExternalInput")
        We18 = nc.dram_tensor("We18", [E, P, kf8 * H1], FP8,
                              kind="ExternalInput")
    out = nc.dram_tensor("out", [T, bc, H2], F32, kind="ExternalOutput")

    with tile.TileContext(nc) as tc:
        with (
            tc.tile_pool(name="persist", bufs=1) as pp,
            tc.tile_pool(name="w1", bufs=4) as w1p,
            tc.tile_pool(name="w1e", bufs=3) as w1ep,
            tc.tile_pool(name="w2", bufs=2) as w2p,
            tc.tile_pool(name="h1", bufs=1) as h1p,
            tc.tile_pool(name="tmp", bufs=2) as tmpp,
            tc.tile_pool(name="small", bufs=2) as smp,
            tc.tile_pool(name="psA", bufs=psa, space="PSUM") as psA,
            tc.tile_pool(name="psB", bufs=psb, space="PSUM") as psB,
        ):
            for _rep in range(reps):
                # ---- resident x (transposed); per-dtile DMAs so the first
                # matmuls start after 1/16 of the load.  The first gate
                # weight pair is DMA'd BEFORE xt so PE can start as soon as
                # xt[d=0] lands. ----
                xt = pp.tile([P, KT * bc], io_dt, tag="xt")
                xtv = xt[:].rearrange("p (n b) -> p n b", b=bc)

                pairs = [(t, g) for t in range(T) for g in range(GT)]
                w1vs = {}

                def load_gate_w1(t, g):
                    w1 = w1p.tile([P, KT * P], io_dt, tag="w1",
                                  name=f"w1g_{t}_{g}")
                    w1v = w1[:].rearrange("p (n h) -> p n h", h=P)
                    nc.sync.dma_start(
                        w1v,
                        Wg1.ap()[t].rearrange("(n p) g -> p n g", p=P)
                        [:, :, g * P:(g + 1) * P])
                    w1vs[(t, g)] = w1v

                # first gate weight, then xt tiles with the other two first-
                # group weights interleaved, so PE starts at ~xt[d0] arrival
                load_gate_w1(*pairs[0])
                xTv = xT.ap().rearrange("(n p) b -> n p b", p=P)
                # remaining gate weights interleaved into the xt stream so
                # later chain groups never stall on queued weight DMAs
                w1_at = {0: pairs[1], 1: pairs[2], 4: pairs[3]}
                for d in range(KT):
                    nc.sync.dma_start(xtv[:, d, :], xTv[d])
                    if d in w1_at:
                        load_gate_w1(*w1_at[d])

                ones = pp.tile([1, P], io_dt, tag="ones")
                nc.sync.dma_start(ones[:], ones_d[:])

                if kf8:
                    xt8 = pp.tile([P, kf8 * bc], FP8, tag="xt8")
                    nc.sync.dma_start(xt8[:], xT8.ap())
                    xt8v = xt8[:].rearrange("p (kp two b) -> p kp two b",
                                            two=2, b=bc)

                # all experts' biases in one DMA each
                be1a = pp.tile([P, E * HT], F32, tag="be1a")
                be1av = be1a[:].rearrange("p (e n) -> p e n", n=HT)
                nc.sync.dma_start(
                    be1av, be1.ap().rearrange("e (n p) -> p e n", p=P))
                be2a = pp.tile([1, E * H2], io_dt, tag="be2a")
                be2av = be2a[:].rearrange("q (e o) -> q e o", o=H2)
                nc.sync.dma_start(be2av, be2.ap()[None, :, :])

                # ---- gate phase ----
                # Both tasks' gate-hidden activations live in one h1-pool
                # slot [P, T, GT, bc].  Chains run >=4-wide with the d-loop
                # OUTER so early matmuls track the xt tiles as they arrive
                # from HBM instead of stalling on the full load.
                bgta = smp.tile([P, T * GT], F32, tag="bg")
                bgtav = bgta[:].rearrange("p (t n) -> p t n", n=GT)
                nc.sync.dma_start(
                    bgtav, bg1.ap().rearrange("t (n p) -> p t n", p=P))
                bgts = [bgtav[:, t, :] for t in range(T)]
                wgsa = smp.tile([P, T * GT * EC], io_dt, tag="wgs")
                wgsav = wgsa[:].rearrange("p (t n e) -> p t n e", n=GT, e=EC)
                nc.sync.dma_start(
                    wgsav, Wgs.ap().rearrange("t (n p) e -> p t n e", p=P))
                wgsvs = [wgsav[:, t] for t in range(T)]

                gt_all = h1p.tile([P, T * GT * bc], io_dt, tag="h1T")
                gtv = gt_all[:].rearrange("p (t n b) -> p t n b", n=GT, b=bc)
                # group 1 is 3 pairs (6 concurrent chains, psA+psB) to keep
                # PE fed while xt streams in; then 3 + 2 pairs
                groups = [pairs[0:3], pairs[3:6], pairs[6:8]]
                for grp in groups:
                    for (t, g) in grp:
                        if (t, g) not in w1vs:
                            load_gate_w1(t, g)
                    chains = [(t, g, cb) for (t, g) in grp
                              for cb in range(NB)]
                    pss = {}
                    for i, c in enumerate(chains):
                        pool = psA if i < 4 else psB
                        pss[c] = pool.tile(
                            [P, bch], F32,
                            tag="psA" if i < 4 else "psB",
                            name=f"psg_{c[0]}_{c[1]}_{c[2]}")
                    for d in range(KT):
                        for (t, g, cb) in chains:
                            nc.tensor.matmul(
                                pss[(t, g, cb)][:],
                                w1vs[(t, g)][:, d, :],
                                xtv[:, d, cb * bch:(cb + 1) * bch],
                                start=(d == 0), stop=(d == KT - 1))
                    for (t, g, cb) in chains:
                        nc.scalar.activation(
                            gtv[:, t, g, cb * bch:(cb + 1) * bch],
                            pss[(t, g, cb)][:], Relu, scale=psc,
                            bias=bgts[t][:, g:g + 1])

                # ---- gate logits + softmax ----
                # all 16 (t, bt) logit chains live in ONE shared psum bank
                # (sub-bank column slices), so the tiny 4-col matmuls run
                # 16-wide and never serialize on PE/SBUF access latency.
                gates = []
                gatesvs = []
                for t in range(T):
                    gates_t = pp.tile([P, NBT * EC], F32, tag=f"gates{t}")
                    gatesvs.append(gates_t[:].rearrange("p (n e) -> p n e",
                                                        e=EC))
                for t in range(T):
                    gatesv = gatesvs[t]
                    for bt0 in range(0, NBT, 2):
                        bts = [bt0, bt0 + 1]
                        pszs = [psB.tile([P, EC], F32, tag="psB",
                                         name=f"psz_{t}_{bt}")
                                for bt in bts]
                        for g in range(GT):
                            for i in range(2):
                                nc.tensor.matmul(
                                    pszs[i][:],
                                    gtv[:, t, g, bts[i] * P:(bts[i] + 1) * P],
                                    wgsvs[t][:, g, :],
                                    start=(g == 0), stop=(g == GT - 1))
                        for i, bt in enumerate(bts):
                            psz = pszs[i]
                            mx = smp.tile([P, 1], F32, tag="mx")
                            nc.vector.reduce_max(mx[:], psz[:], axis=AxX)
                            sh = smp.tile([P, EC], F32, tag="sh")
                            nc.vector.tensor_scalar_sub(sh[:], psz[:], mx[:])
                            ex = smp.tile([P, EC], F32, tag="ex")
                            ssum = smp.tile([P, 1], F32, tag="ss")
                            nc.scalar.activation(ex[:], sh[:], Exp,
                                                 accum_out=ssum[:])
                            rec = smp.tile([P, 1], F32, tag="rc")
                            nc.vector.reciprocal(rec[:], ssum[:])
                            nc.vector.tensor_scalar_mul(gatesv[:, bt, :],
                                                        ex[:], rec[:])
                gates = gatesvs

                # ---- output accumulators ----
                accs = []
                for t in range(T):
                    acc = pp.tile([P, NBT * H2], F32, tag=f"acc{t}")
                    accs.append(acc[:].rearrange("p (n o) -> p n o", o=H2))

                # ---- expert loop ----
                for e in range(E):
                    w2t = w2p.tile([P, HT * H2], io_dt, tag="w2")
                    w2v = w2t[:].rearrange("p (n o) -> p n o", o=H2)
                    nc.sync.dma_start(
                        w2v, We2.ap()[e].rearrange("(n p) o -> p n o", p=P))
                    w2vs = [w2v[:, ht, :] for ht in range(HT)]
                    be2t = be2av[:, e, :]
                    be1t = be1av[:, e, :]
                    if kf8:
                        w18 = w2p.tile([P, kf8 * H1], FP8, tag="w18")
                        nc.sync.dma_start(w18[:], We18.ap()[e])
                        w18v = w18[:].rearrange(
                            "p (kp two h) -> p kp two h", two=2, h=H1)

                    # be2 broadcast to [P, H2] once per expert (one K=1
                    # ones-matmul + copy), DVE-added into each L2 psum below
                    psb2 = psB.tile([P, H2], F32, tag="psB")
                    nc.tensor.matmul(psb2[:], ones[:], be2t,
                                     start=True, stop=True)
                    be2b = smp.tile([P, H2], F32, tag="be2b")
                    nc.scalar.copy(be2b[:], psb2[:])

                    h1 = h1p.tile([P, HT * bc], io_dt, tag="h1T")
                    h1v = h1[:].rearrange("p (n b) -> p n b", b=bc)

                    # L1: h1T[h, b] = relu(sum_d We1[d, h]^T x[d, b] + be1)
                    # expert weights arrive in two half-K DMAs (not 8); the
                    # NB b-chunk chains interleave per-d so consecutive
                    # matmuls share the same stationary weights and chain
                    # boundaries overlap (psA double-buffers 2 ht groups)
                    KH = KT // 2
                    w1evs = []
                    for half in range(2):
                        w1e = w1ep.tile([P, KH * H1], io_dt, tag="w1e")
                        w1ev = w1e[:].rearrange("p (n h) -> p n h", h=H1)
                        nc.sync.dma_start(
                            w1ev,
                            We1.ap()[e].rearrange("(n p) h -> p n h", p=P)
                            [:, half * KH:(half + 1) * KH, :])
                        w1evs.append(w1ev)
                    for ht in range(HT):
                        pss1 = [psA.tile([P, bch], F32, tag="psA",
                                         name=f"ps1_{e}_{ht}_{cb}")
                                for cb in range(NB)]
                        # chain: bf16 d=kf8 first (full-width start so the
                        # half-width fp8 writes never re-zero the bank),
                        # then the fp8 DoubleRow pairs, then bf16 rest
                        d0 = kf8
                        for cb in range(NB):
                            nc.tensor.matmul(
                                pss1[cb][:],
                                w1evs[d0 // KH][:, d0 % KH,
                                                ht * P:(ht + 1) * P],
                                xtv[:, d0, cb * bch:(cb + 1) * bch],
                                start=True, stop=False)
                        for kp in range(kf8 // 2):
                            for cb in range(NB):
                                for hf in range(bch // 256):
                                    off = cb * bch + hf * 256
                                    nc.tensor.matmul(
                                        pss1[cb][:, hf * 256:(hf + 1) * 256],
                                        w18v[:, kp, :, ht * P:(ht + 1) * P],
                                        xt8v[:, kp, :, off:off + 256],
                                        start=False, stop=False,
                                        perf_mode=DRMODE,
                                        skip_group_check=True)
                        for d in range(d0 + 1, KT):
                            for cb in range(NB):
                                nc.tensor.matmul(
                                    pss1[cb][:],
                                    w1evs[d // KH][:, d % KH,
                                                   ht * P:(ht + 1) * P],
                                    xtv[:, d, cb * bch:(cb + 1) * bch],
                                    start=False, stop=(d == KT - 1))
                        for cb in range(NB):
                            nc.scalar.activation(
                                h1v[:, ht, cb * bch:(cb + 1) * bch],
                                pss1[cb][:], Relu, scale=psc,
                                bias=be1t[:, ht:ht + 1])

                    # L2 + gated accumulation; btile pairs interleave so
                    # consecutive matmuls share the moving w2 operand and
                    # chain boundaries overlap (psB double-buffers)
                    for bt0 in range(0, NBT, 2):
                        bts = [bt0, bt0 + 1]
                        pss2 = [psB.tile([P, H2], F32, tag="psB",
                                         name=f"ps2_{e}_{bt}")
                                for bt in bts]
                        for ht in range(HT):
                            for i, bt in enumerate(bts):
                                nc.tensor.matmul(
                                    pss2[i][:],
                                    h1v[:, ht, bt * P:(bt + 1) * P],
                                    w2vs[ht],
                                    start=(ht == 0), stop=(ht == HT - 1))
                        for i, bt in enumerate(bts):
                            ps2 = pss2[i]
                            nc.vector.tensor_add(ps2[:], ps2[:], be2b[:])
                            for t in range(T):
                                if e not in TASK_EXPERTS[t]:
                                    continue
                                j = TASK_EXPERTS[t].index(e)
                                gate_ap = gates[t][:, bt, j:j + 1]
                                if e == TASK_EXPERTS[t][0]:
                                    nc.scalar.activation(
                                        accs[t][:, bt, :], ps2[:], Relu,
                                        scale=gate_ap)
                                else:
                                    tmp = tmpp.tile([P, H2], F32, tag="tmp")
                                    nc.scalar.activation(tmp[:], ps2[:],
                                                         Relu, scale=gate_ap)
                                    nc.vector.tensor_add(accs[t][:, bt, :],
                                                         accs[t][:, bt, :],
                                                         tmp[:])
                                if e == TASK_EXPERTS[t][-1]:
                                    nc.sync.dma_start(
                                        out.ap()[t].rearrange(
                                            "(n p) o -> p n o", p=P)
                                        [:, bt, :],
                                        accs[t][:, bt, :])

    nc.compile()
    return nc


# ---------------------------------------------------------------------------
# host-side SPMD execution (mirrors bass_utils.run_bass_kernel_spmd's axon
# path, but keeps the jitted callable so repeat calls don't recompile)
# ---------------------------------------------------------------------------
class SpmdRunner:
    def __init__(self, nc, n_cores):
        import jax
        from jax.sharding import Mesh, PartitionSpec
        from jax.experimental.shard_map import shard_map
        from concourse.bass2jax import (_bass_exec_p, install_neuronx_cc_hook,
                                        partition_id_tensor)
        install_neuronx_cc_hook()
        self.jax = jax
        self.nc = nc
        self.n_cores = n_cores
        partition_name = (nc.partition_id_tensor.name
                          if nc.partition_id_tensor else None)
        in_names, out_names, out_avals, zero_outs = [], [], [], []
        for alloc in nc.m.functions[0].allocations:
            if not isinstance(alloc, mybir.MemoryLocationSet):
                continue
            name = alloc.memorylocations[0].name
            if alloc.kind == "ExternalInput":
                if name != partition_name:
                    in_names.append(name)
            elif alloc.kind == "ExternalOutput":
                out_names.append(name)
                shape = tuple(alloc.tensor_shape)
                dtype = mybir.dt.np(alloc.dtype)
                out_avals.append(jax.core.ShapedArray(shape, dtype))
                zero_outs.append(np.zeros(shape, dtype))
        all_in_names = list(in_names) + list(out_names)
        if partition_name is not None:
            all_in_names.append(partition_name)

        def _body(*args):
            operands = list(args)
            if partition_name is not None:
                operands.append(partition_id_tensor())
            outs = _bass_exec_p.bind(
                *operands,
                out_avals=tuple(out_avals),
                in_names=tuple(all_in_names),
                out_names=tuple(out_names),
                lowering_input_output_aliases=(),
                sim_require_finite=True,
                sim_require_nnan=True,
                nc=nc,
            )
            return tuple(outs)

        devices = jax.devices()[:n_cores]
        assert len(devices) == n_cores
        self.mesh = Mesh(np.asarray(devices), ("core",))
        n_args = len(in_names) + len(out_names)
        self.fn = jax.jit(
            shard_map(_body, mesh=self.mesh,
                      in_specs=(PartitionSpec("core"),) * n_args,
                      out_specs=(PartitionSpec("core"),) * len(out_names),
                      check_rep=False),
            keep_unused=True,
        )
        self.in_names = in_names
        self.out_names = out_names
        self.out_avals = out_avals
        self.zero_outs = zero_outs
        self.PartitionSpec = PartitionSpec

    def put_inputs(self, in_maps):
        jax = self.jax
        concat_in = [
            np.concatenate([np.asarray(m[name]) for m in in_maps], axis=0)
            for name in self.in_names
        ]
        concat_zeros = [
            np.zeros((self.n_cores * z.shape[0], *z.shape[1:]), z.dtype)
            for z in self.zero_outs
        ]
        sh = jax.sharding.NamedSharding(self.mesh, self.PartitionSpec("core"))
        return [jax.device_put(a, sh) for a in concat_in + concat_zeros]

    def run(self, args):
        out = self.fn(*args)
        self.jax.block_until_ready(out)
        return out

    def results(self, out_arrs):
        return [
            {name: np.asarray(out_arrs[i]).reshape(
                self.n_cores, *self.out_avals[i].shape)[c]
             for i, name in enumerate(self.out_names)}
            for c in range(self.n_cores)
        ]


_CACHE = {}


def _to_io(a, mm_mode):
    a = np.asarray(a, np.float32)
    if mm_mode == "bf16":
        import ml_dtypes
        return np.ascontiguousarray(a.astype(ml_dtypes.bfloat16))
    return np.ascontiguousarray(a)


def _fp8_swizzle(a, scale, kf8):
    """[kf8*128, N] f32 -> fp8 device layout [128, kf8//2, 2, N] flat."""
    import ml_dtypes
    q = np.asarray(a[:kf8 * P] * scale, ml_dtypes.float8_e4m3)
    n = q.shape[1]
    return np.ascontiguousarray(
        q.reshape(kf8 // 2, 2, P, n).transpose(2, 0, 1, 3).reshape(P, -1))


def make_in_maps(x, We1, be1, We2, be2, Wg1, bg1, Wgs, mm_mode,
                 kf8=None):
    if kf8 is None:
        kf8 = KF8_DEFAULT if mm_mode == "bf16" else 0
    bs = BS if mm_mode == "bf16" else 1.0
    We1f = np.asarray(We1, np.float32)
    shared = {
        "We1": _to_io(We1f * bs, mm_mode),
        "be1": np.ascontiguousarray(np.asarray(be1, np.float32)),
        "We2": _to_io(We2, mm_mode),
        "be2": _to_io(be2, mm_mode),
        "Wg1": _to_io(np.asarray(Wg1, np.float32) * bs, mm_mode),
        "bg1": np.ascontiguousarray(np.asarray(bg1, np.float32)),
        "Wgs": _to_io(Wgs, mm_mode),
        "ones": _to_io(np.ones((1, P), np.float32), mm_mode),
    }
    if kf8:
        shared["We18"] = np.ascontiguousarray(np.stack(
            [_fp8_swizzle(We1f[e], WS8, kf8) for e in range(E)]))
    x = np.asarray(x, np.float32)
    in_maps = []
    for c in range(NCORES):
        xs = x[c * BC:(c + 1) * BC]
        m = {"xT": _to_io(xs.T * bs, mm_mode), **shared}
        if kf8:
            m["xT8"] = _fp8_swizzle(xs.T, XS8, kf8)
        in_maps.append(m)
    return in_maps


def get_runner(mm_mode="bf16", reps=1):
    key = (mm_mode, reps)
    if key not in _CACHE:
        nc = build_program(reps=reps, mm_mode=mm_mode)
        _CACHE[key] = SpmdRunner(nc, NCORES)
    return _CACHE[key]


MM_MODE = "bf16"


def kernel(x, We1, be1, We2, be2, Wg1, bg1, Wgs):
    runner = get_runner(MM_MODE)
    in_maps = make_in_maps(x, We1, be1, We2, be2, Wg1, bg1, Wgs, MM_MODE)
    args = runner.put_inputs(in_maps)
    res = runner.results(runner.run(args))
    out = np.concatenate([r["out"] for r in res], axis=1)  # [T, B, H2]
    return np.ascontiguousarray(out.astype(np.float32))



# revision 15
# speedup vs baseline: 1.0589x; 1.0589x over previous
"""Trainium2 Bass kernel for the CGC (multi-task MoE) layer.

Reference computation (all-dense MoE, T=2 tasks, E=6 experts, EC=4 per task):
    h1 = relu(x @ We1[e] + be1[e])            [B, E, H1]
    h2 = relu(h1 @ We2[e] + be2[e])           [B, E, H2]
    g  = relu(x @ Wg1[t] + bg1[t])            [B, T, G]
    gate = softmax(g @ Wgs[t])                [B, T, EC]
    out[t, b, :] = sum_j gate[b, t, j] * h2[b, IDX[t, j], :]

Sharding: data-parallel over batch across 8 NeuronCores (B=8192 -> 1024
rows/core), weights replicated, no collectives.  The host pre-transposes each
x shard to xT[D, BC] so every matmul's contraction dim sits on SBUF
partitions with no on-device transposes:

    L1:  psum[h, b] += We1[d, h].T-block @ xT[d, b]      (lhsT=We1, rhs=xT)
    L2:  psum[b, o] += h1T[h, b].T-block @ We2[h, o]     (lhsT=h1T, rhs=We2)

Quantization plan (error budget: rel_err < 2e-2 vs the reference):
  - L1 expert matmul: first KF8=8 of 16 d-tiles as fp8e4 DoubleRow pairs,
    rest fp16.  fp8 scales x*4 / W*1024, fp16 x*64 / W*64: all products land
    in PSUM at scale 4096, descaled in the relu activation.
  - gate / L2 / logits stay fp16 (same PE rate and bytes as bf16, 8x finer
    mantissa; measured rel err 1.93e-2 vs 2.17e-2 at KF8=10 and 2.03-2.08e-2
    with an fp8 gate, whose error is heavy-tailed in the absmax metric).
fp8 DoubleRow streams ~1.13 cyc/row covering 2 k-tiles -> ~1.8x bf16 rate.
be2 is folded into the L2 PSUM accumulation with a K=1 ones-matmul.
relu+gate scaling is fused on ScalarE (gate > 0 so relu(g*x) = g*relu(x)).
"""

import contextlib

import numpy as np

import concourse.bass as bass
import concourse.mybir as mybir
import concourse.tile as tile
from concourse import bacc

# ---- problem shapes (hardcoded per contest contract) ----
B, D, H1, H2, G = 8192, 2048, 1024, 512, 512
T, SPEC, SH = 2, 2, 2
E = T * SPEC + SH          # 6
EC = SPEC + SH             # 4
NCORES = 8
BC = B // NCORES           # 1024 per-core batch
TASK_EXPERTS = [[0, 1, 4, 5], [2, 3, 4, 5]]   # IDX rows

F32 = mybir.dt.float32
F32R = mybir.dt.float32r
BF16 = mybir.dt.bfloat16
FP16 = mybir.dt.float16
FP8 = mybir.dt.float8e4
P = 128
KT = D // P                # 16
GT = G // P                # 4
HT = H1 // P               # 8
DRMODE = mybir.MatmulPerfMode.DoubleRow

# fp8 config: gate fully fp8; first KF8 of 16 L1 d-tiles as fp8 DoubleRow
# pairs.  Scales: fp8 ops at x*4 / W*1024, bf16 ops at x*64 / W*64 so every
# product lands in PSUM at scale 4096, descaled in the relu activation.
KF8_DEFAULT = 8
GATE8_DEFAULT = False
XS8, WS8, BS = 4.0, 1024.0, 64.0
PSC = 1.0 / (BS * BS)  # psum descale for the scaled accumulation

Relu = mybir.ActivationFunctionType.Relu
Exp = mybir.ActivationFunctionType.Exp
AxX = mybir.AxisListType.X


def build_program(bc=BC, bch=None, reps=1, mm_mode="bf16", psa=4,
                  psb=4, kf8=KF8_DEFAULT, gate8=GATE8_DEFAULT):
    """Build the SPMD single-core program.

    bc: per-core batch size; bch: L1/gate moving free-dim chunk; reps: body
    repetitions (for wall-clock differencing); mm_mode: f32 | f32r | bf16.
    """
    if bch is None:
        bch = min(512, bc)
    assert bc % P == 0 and bc % bch == 0
    NB, NBT = bc // bch, bc // P

    # matmul-operand dtype: walrus requires f32r matmul inputs to be produced
    # as f32r, so the whole producer chain (DRAM tensor -> DMA -> tile) is
    # declared f32r in that mode (same bytes as f32 host-side).  f16 runs at
    # the same PE rate and byte width as bf16 but with 8x finer mantissa.
    io_dt = {"bf16": BF16, "f16": FP16, "f32r": F32R, "f32": F32}[mm_mode]
    scaled = mm_mode in ("bf16", "f16")  # x/We1/Wg1 arrive pre-scaled by 64
    if not scaled:
        kf8 = 0
        gate8 = False
    psc = PSC if scaled else 1.0
    assert kf8 % 2 == 0
    KB = KT - kf8              # bf16 d-tiles (L1)
    KH = max(KB // 2, 1)       # bf16 weight tiles per half-DMA
    # bf16 x tiles resident on device: trimmed to the L1 remainder when the
    # gate path is fp8, else the full K (the bf16 gate needs all 16)
    XOFF = kf8 if gate8 else 0
    XTN = KT - XOFF
    XT8N = KT if gate8 else kf8    # fp8 x tiles resident (gate uses all 16)

    nc = bacc.Bacc("TRN2", target_bir_lowering=False, debug=False,
                   num_devices=NCORES)

    # bf16 x / We1 carry only the d-tiles not covered by fp8
    xT = nc.dram_tensor("xT", [XTN * P, bc], io_dt, kind="ExternalInput")
    We1 = nc.dram_tensor("We1", [E, KB * P, H1], io_dt, kind="ExternalInput")
    be1 = nc.dram_tensor("be1", [E, H1], F32, kind="ExternalInput")
    We2 = nc.dram_tensor("We2", [E, H1, H2], io_dt, kind="ExternalInput")
    be2 = nc.dram_tensor("be2", [E, H2], io_dt, kind="ExternalInput")
    if not gate8:
        Wg1 = nc.dram_tensor("Wg1", [T, D, G], io_dt, kind="ExternalInput")
    bg1 = nc.dram_tensor("bg1", [T, G], F32, kind="ExternalInput")
    Wgs = nc.dram_tensor("Wgs", [T, G, EC], io_dt, kind="ExternalInput")
    # K=1 all-ones lhsT used to broadcast be2 into the L2 PSUM accumulation;
    # an input tensor because only DMA can produce f32r-tagged data.
    ones_d = nc.dram_tensor("ones", [1, P], io_dt, kind="ExternalInput")
    if kf8 or gate8:
        # pre-swizzled host layout [p, kp, two, .] flattened; x carries all
        # 16 d-tiles (gate uses all of them, L1 the first kf8)
        xT8 = nc.dram_tensor("xT8", [P, XT8N * bc], FP8,
                             kind="ExternalInput")
    if kf8:
        We18 = nc.dram_tensor("We18", [E, P, kf8 * H1], FP8,
                              kind="ExternalInput")
    if gate8:
        Wg18 = nc.dram_tensor("Wg18", [T, P, KT * G], FP8,
                              kind="ExternalInput")
    out = nc.dram_tensor("out", [T, bc, H2], F32, kind="ExternalOutput")

    with tile.TileContext(nc) as tc, contextlib.ExitStack() as stack:
        ep = stack.enter_context
        pp = ep(tc.tile_pool(name="persist", bufs=1))
        # xt8 is read until the end of the expert loop; double-buffering it
        # lets the next rep's gate phase start before this rep fully drains
        xbp = ep(tc.tile_pool(name="xb", bufs=2))
        w1p = None if gate8 else ep(tc.tile_pool(name="w1", bufs=4))
        w1ep = ep(tc.tile_pool(name="w1e", bufs=3))
        w2p = ep(tc.tile_pool(name="w2", bufs=2))
        h1p = ep(tc.tile_pool(name="h1", bufs=2))
        tmpp = ep(tc.tile_pool(name="tmp", bufs=2))
        smp = ep(tc.tile_pool(name="small", bufs=2))
        psA = ep(tc.tile_pool(name="psA", bufs=psa, space="PSUM"))
        psB = ep(tc.tile_pool(name="psB", bufs=psb, space="PSUM"))

        for _rep in range(reps):
            # ---- resident x: fp8 (all d-tiles, per-kp DMAs) + bf16
            # remainder.  Gate weights land first so PE starts as soon as
            # xt8[kp=0] arrives. ----
            if gate8:
                wg18 = pp.tile([P, T * KT * G], FP8, tag="wg18")
                wg18v = wg18[:].rearrange(
                    "p (t kp two g) -> p t kp two g", t=T, two=2, g=G)
                nc.sync.dma_start(
                    wg18v[:, 0], Wg18.ap()[0].rearrange(
                        "p (kp two g) -> p kp two g", two=2, g=G))
            if kf8 or gate8:
                xt8 = xbp.tile([P, XT8N * bc], FP8, tag="xt8")
                xt8v = xt8[:].rearrange(
                    "p (kp two b) -> p kp two b", two=2, b=bc)
                xT8v = xT8.ap().rearrange(
                    "p (kp two b) -> p kp two b", two=2, b=bc)
                for kp in range(XT8N // 2):
                    nc.sync.dma_start(xt8v[:, kp], xT8v[:, kp])
                    if gate8 and kp == 1:
                        nc.sync.dma_start(
                            wg18v[:, 1], Wg18.ap()[1].rearrange(
                                "p (kp two g) -> p kp two g", two=2, g=G))

            pairs = [(t, gg) for t in range(T) for gg in range(GT)]
            w1vs = {}

            def load_gate_w1(t, gg):
                w1 = w1p.tile([P, KT * P], io_dt, tag="w1",
                              name=f"w1g_{t}_{gg}")
                w1v = w1[:].rearrange("p (n h) -> p n h", h=P)
                nc.sync.dma_start(
                    w1v,
                    Wg1.ap()[t].rearrange("(n p) g -> p n g", p=P)
                    [:, :, gg * P:(gg + 1) * P])
                w1vs[(t, gg)] = w1v

            if not gate8:
                load_gate_w1(*pairs[0])

            # bf16 x remainder (the L1 d-tiles >= kf8), per-d DMAs
            xt = pp.tile([P, XTN * bc], io_dt, tag="xt")
            xtv = xt[:].rearrange("p (n b) -> p n b", b=bc)
            xTv = xT.ap().rearrange("(n p) b -> n p b", p=P)
            w1_at = {} if gate8 else {0: pairs[1], 1: pairs[2], 4: pairs[3]}
            for d in range(XTN):
                nc.sync.dma_start(xtv[:, d, :], xTv[d])
                if d in w1_at:
                    load_gate_w1(*w1_at[d])
            # L1's bf16 remainder lives at this offset inside xtv
            XL1 = kf8 - XOFF

            ones = pp.tile([1, P], io_dt, tag="ones")
            nc.sync.dma_start(ones[:], ones_d[:])

            # all experts' biases in one DMA each
            be1a = pp.tile([P, E * HT], F32, tag="be1a")
            be1av = be1a[:].rearrange("p (e n) -> p e n", n=HT)
            nc.sync.dma_start(
                be1av, be1.ap().rearrange("e (n p) -> p e n", p=P))
            be2a = pp.tile([1, E * H2], io_dt, tag="be2a")
            be2av = be2a[:].rearrange("q (e o) -> q e o", o=H2)
            nc.sync.dma_start(be2av, be2.ap()[None, :, :])

            # ---- gate phase ----
            # Both tasks' gate-hidden activations live in one h1-pool slot
            # [P, T, GT, bc].  Chains run >=4-wide with the contraction loop
            # OUTER so early matmuls track the x tiles arriving from HBM.
            bgta = smp.tile([P, T * GT], F32, tag="bg")
            bgtav = bgta[:].rearrange("p (t n) -> p t n", n=GT)
            nc.sync.dma_start(
                bgtav, bg1.ap().rearrange("t (n p) -> p t n", p=P))
            bgts = [bgtav[:, t, :] for t in range(T)]
            wgsa = smp.tile([P, T * GT * EC], io_dt, tag="wgs")
            wgsav = wgsa[:].rearrange("p (t n e) -> p t n e", n=GT, e=EC)
            nc.sync.dma_start(
                wgsav, Wgs.ap().rearrange("t (n p) e -> p t n e", p=P))
            wgsvs = [wgsav[:, t] for t in range(T)]

            gt_all = h1p.tile([P, T * GT * bc], io_dt, tag="h1T")
            gtv = gt_all[:].rearrange("p (t n b) -> p t n b", n=GT, b=bc)

            gatesvs = []
            for t in range(T):
                gates_t = pp.tile([P, NBT * EC], F32, tag=f"gates{t}")
                gatesvs.append(gates_t[:].rearrange("p (n e) -> p n e",
                                                    e=EC))

            def emit_logits(t):
                # tiny 4-col logit matmuls share one psum bank (column
                # slices) so they pipeline; softmax runs on DVE/ScalarE
                # underneath the remaining PE stream.
                gatesv = gatesvs[t]
                for bt0 in range(0, NBT, 2):
                    bts = [bt0, bt0 + 1]
                    pszs = [psB.tile([P, EC], F32, tag="psB",
                                     name=f"psz_{t}_{bt}")
                            for bt in bts]
                    for gg in range(GT):
                        for i in range(2):
                            nc.tensor.matmul(
                                pszs[i][:],
                                gtv[:, t, gg,
                                    bts[i] * P:(bts[i] + 1) * P],
                                wgsvs[t][:, gg, :],
                                start=(gg == 0), stop=(gg == GT - 1))
                    for i, bt in enumerate(bts):
                        psz = pszs[i]
                        mx = smp.tile([P, 1], F32, tag="mx")
                        nc.vector.reduce_max(mx[:], psz[:], axis=AxX)
                        sh = smp.tile([P, EC], F32, tag="sh")
                        nc.vector.tensor_scalar_sub(sh[:], psz[:], mx[:])
                        ex = smp.tile([P, EC], F32, tag="ex")
                        ssum = smp.tile([P, 1], F32, tag="ss")
                        nc.scalar.activation(ex[:], sh[:], Exp,
                                             accum_out=ssum[:])
                        rec = smp.tile([P, 1], F32, tag="rc")
                        nc.vector.reciprocal(rec[:], ssum[:])
                        nc.vector.tensor_scalar_mul(gatesv[:, bt, :],
                                                    ex[:], rec[:])

            # groups of (t, g) pairs; all of t=0's hiddens exist after group
            # 1 and t=1's after group 2, so each task's logits+softmax issue
            # early and overlap the remaining matmul stream.
            groups = [pairs[0:3], pairs[3:6], pairs[6:8]]
            for gi, grp in enumerate(groups):
                if not gate8:
                    for (t, gg) in grp:
                        if (t, gg) not in w1vs:
                            load_gate_w1(t, gg)
                chains = [(t, gg, cb) for (t, gg) in grp
                          for cb in range(NB)]
                pss = {}
                for i, c in enumerate(chains):
                    pool = psA if i < 4 else psB
                    pss[c] = pool.tile(
                        [P, bch], F32,
                        tag="psA" if i < 4 else "psB",
                        name=f"psg_{c[0]}_{c[1]}_{c[2]}")
                if gate8:
                    for kp in range(KT // 2):
                        for (t, gg, cb) in chains:
                            nc.tensor.matmul(
                                pss[(t, gg, cb)][:],
                                wg18v[:, t, kp, :, gg * P:(gg + 1) * P],
                                xt8v[:, kp, :, cb * bch:(cb + 1) * bch],
                                start=(kp == 0), stop=(kp == KT // 2 - 1),
                                perf_mode=DRMODE, skip_group_check=True)
                else:
                    for d in range(KT):
                        for (t, gg, cb) in chains:
                            nc.tensor.matmul(
                                pss[(t, gg, cb)][:],
                                w1vs[(t, gg)][:, d, :],
                                xtv[:, d, cb * bch:(cb + 1) * bch],
                                start=(d == 0), stop=(d == KT - 1))
                for (t, gg, cb) in chains:
                    nc.scalar.activation(
                        gtv[:, t, gg, cb * bch:(cb + 1) * bch],
                        pss[(t, gg, cb)][:], Relu, scale=psc,
                        bias=bgts[t][:, gg:gg + 1])
                if gi == 1:
                    emit_logits(0)
                if gi == 2:
                    emit_logits(1)

            gates = gatesvs

            # ---- output accumulators ----
            accs = []
            for t in range(T):
                acc = pp.tile([P, NBT * H2], F32, tag=f"acc{t}")
                accs.append(acc[:].rearrange("p (n o) -> p n o", o=H2))

            # ---- expert loop ----
            n_seen = [0, 0]
            for e in range(E):
                w2t = w2p.tile([P, HT * H2], io_dt, tag="w2")
                w2v = w2t[:].rearrange("p (n o) -> p n o", o=H2)
                nc.sync.dma_start(
                    w2v, We2.ap()[e].rearrange("(n p) o -> p n o", p=P))
                w2vs = [w2v[:, ht, :] for ht in range(HT)]
                be2t = be2av[:, e, :]
                be1t = be1av[:, e, :]
                if kf8:
                    w18 = w2p.tile([P, kf8 * H1], FP8, tag="w18")
                    nc.sync.dma_start(w18[:], We18.ap()[e])
                    w18v = w18[:].rearrange(
                        "p (kp two h) -> p kp two h", two=2, h=H1)

                # be2 broadcast to [P, H2] once per expert (one K=1
                # ones-matmul + copy), DVE-added into each L2 psum below
                psb2 = psB.tile([P, H2], F32, tag="psB")
                nc.tensor.matmul(psb2[:], ones[:], be2t,
                                 start=True, stop=True)
                be2b = smp.tile([P, H2], F32, tag="be2b")
                nc.scalar.copy(be2b[:], psb2[:])

                h1 = h1p.tile([P, HT * bc], io_dt, tag="h1T")
                h1v = h1[:].rearrange("p (n b) -> p n b", b=bc)

                # L1: h1T[h, b] = relu(sum_d We1[d, h]^T x[d, b] + be1)
                # bf16 expert weights arrive in two half-K DMAs; the NB
                # b-chunk chains interleave per-d so consecutive matmuls
                # share the same stationary weights and chain boundaries
                # overlap (psA double-buffers 2 ht groups)
                w1evs = []
                for half in range(2):
                    w1e = w1ep.tile([P, KH * H1], io_dt, tag="w1e")
                    w1ev = w1e[:].rearrange("p (n h) -> p n h", h=H1)
                    nc.sync.dma_start(
                        w1ev,
                        We1.ap()[e].rearrange("(n p) h -> p n h", p=P)
                        [:, half * KH:(half + 1) * KH, :])
                    w1evs.append(w1ev)
                for ht in range(HT):
                    pss1 = [psA.tile([P, bch], F32, tag="psA",
                                     name=f"ps1_{e}_{ht}_{cb}")
                            for cb in range(NB)]
                    # chain: first bf16 remainder tile opens the bank
                    # full-width, then the fp8 DoubleRow pairs, then the
                    # rest of the bf16 tiles
                    for cb in range(NB):
                        nc.tensor.matmul(
                            pss1[cb][:],
                            w1evs[0][:, 0, ht * P:(ht + 1) * P],
                            xtv[:, XL1, cb * bch:(cb + 1) * bch],
                            start=True, stop=False)
                    for kp in range(kf8 // 2):
                        for cb in range(NB):
                            nc.tensor.matmul(
                                pss1[cb][:],
                                w18v[:, kp, :, ht * P:(ht + 1) * P],
                                xt8v[:, kp, :, cb * bch:(cb + 1) * bch],
                                start=False, stop=False,
                                perf_mode=DRMODE,
                                skip_group_check=True)
                    for d in range(1, KB):
                        for cb in range(NB):
                            nc.tensor.matmul(
                                pss1[cb][:],
                                w1evs[d // KH][:, d % KH,
                                               ht * P:(ht + 1) * P],
                                xtv[:, XL1 + d, cb * bch:(cb + 1) * bch],
                                start=False, stop=(d == KB - 1))
                    for cb in range(NB):
                        nc.scalar.activation(
                            h1v[:, ht, cb * bch:(cb + 1) * bch],
                            pss1[cb][:], Relu, scale=psc,
                            bias=be1t[:, ht:ht + 1])

                # L2 + gated accumulation; btile pairs interleave so
                # consecutive matmuls share the moving w2 operand and chain
                # boundaries overlap (psB double-buffers)
                for t in range(T):
                    if e in TASK_EXPERTS[t]:
                        n_seen[t] += 1
                for bt0 in range(0, NBT, 2):
                    bts = [bt0, bt0 + 1]
                    pss2 = [psB.tile([P, H2], F32, tag="psB",
                                     name=f"ps2_{e}_{bt}")
                            for bt in bts]
                    for ht in range(HT):
                        for i, bt in enumerate(bts):
                            nc.tensor.matmul(
                                pss2[i][:],
                                h1v[:, ht, bt * P:(bt + 1) * P],
                                w2vs[ht],
                                start=(ht == 0), stop=(ht == HT - 1))
                    for i, bt in enumerate(bts):
                        ps2 = pss2[i]
                        nc.vector.tensor_add(ps2[:], ps2[:], be2b[:])
                        for t in range(T):
                            if e not in TASK_EXPERTS[t]:
                                continue
                            j = TASK_EXPERTS[t].index(e)
                            gate_ap = gates[t][:, bt, j:j + 1]
                            if n_seen[t] == 1:
                                nc.scalar.activation(
                                    accs[t][:, bt, :], ps2[:], Relu,
                                    scale=gate_ap)
                            else:
                                tmp = tmpp.tile([P, H2], F32, tag="tmp")
                                nc.scalar.activation(tmp[:], ps2[:],
                                                     Relu, scale=gate_ap)
                                nc.vector.tensor_add(accs[t][:, bt, :],
                                                     accs[t][:, bt, :],
                                                     tmp[:])
                            if n_seen[t] == EC:
                                nc.sync.dma_start(
                                    out.ap()[t].rearrange(
                                        "(n p) o -> p n o", p=P)
                                    [:, bt, :],
                                    accs[t][:, bt, :])

    nc.compile()
    return nc


# ---------------------------------------------------------------------------
# host-side SPMD execution (mirrors bass_utils.run_bass_kernel_spmd's axon
# path, but keeps the jitted callable so repeat calls don't recompile)
# ---------------------------------------------------------------------------
class SpmdRunner:
    def __init__(self, nc, n_cores):
        import jax
        from jax.sharding import Mesh, PartitionSpec
        from jax.experimental.shard_map import shard_map
        from concourse.bass2jax import (_bass_exec_p, install_neuronx_cc_hook,
                                        partition_id_tensor)
        install_neuronx_cc_hook()
        self.jax = jax
        self.nc = nc
        self.n_cores = n_cores
        partition_name = (nc.partition_id_tensor.name
                          if nc.partition_id_tensor else None)
        in_names, out_names, out_avals, zero_outs = [], [], [], []
        for alloc in nc.m.functions[0].allocations:
            if not isinstance(alloc, mybir.MemoryLocationSet):
                continue
            name = alloc.memorylocations[0].name
            if alloc.kind == "ExternalInput":
                if name != partition_name:
                    in_names.append(name)
            elif alloc.kind == "ExternalOutput":
                out_names.append(name)
                shape = tuple(alloc.tensor_shape)
                dtype = mybir.dt.np(alloc.dtype)
                out_avals.append(jax.core.ShapedArray(shape, dtype))
                zero_outs.append(np.zeros(shape, dtype))
        all_in_names = list(in_names) + list(out_names)
        if partition_name is not None:
            all_in_names.append(partition_name)

        def _body(*args):
            operands = list(args)
            if partition_name is not None:
                operands.append(partition_id_tensor())
            outs = _bass_exec_p.bind(
                *operands,
                out_avals=tuple(out_avals),
                in_names=tuple(all_in_names),
                out_names=tuple(out_names),
                lowering_input_output_aliases=(),
                sim_require_finite=True,
                sim_require_nnan=True,
                nc=nc,
            )
            return tuple(outs)

        devices = jax.devices()[:n_cores]
        assert len(devices) == n_cores
        self.mesh = Mesh(np.asarray(devices), ("core",))
        n_args = len(in_names) + len(out_names)
        self.fn = jax.jit(
            shard_map(_body, mesh=self.mesh,
                      in_specs=(PartitionSpec("core"),) * n_args,
                      out_specs=(PartitionSpec("core"),) * len(out_names),
                      check_rep=False),
            keep_unused=True,
        )
        self.in_names = in_names
        self.out_names = out_names
        self.out_avals = out_avals
        self.zero_outs = zero_outs
        self.PartitionSpec = PartitionSpec

    def put_inputs(self, in_maps):
        jax = self.jax
        concat_in = [
            np.concatenate([np.asarray(m[name]) for m in in_maps], axis=0)
            for name in self.in_names
        ]
        concat_zeros = [
            np.zeros((self.n_cores * z.shape[0], *z.shape[1:]), z.dtype)
            for z in self.zero_outs
        ]
        sh = jax.sharding.NamedSharding(self.mesh, self.PartitionSpec("core"))
        return [jax.device_put(a, sh) for a in concat_in + concat_zeros]

    def run(self, args):
        out = self.fn(*args)
        self.jax.block_until_ready(out)
        return out

    def results(self, out_arrs):
        return [
            {name: np.asarray(out_arrs[i]).reshape(
                self.n_cores, *self.out_avals[i].shape)[c]
             for i, name in enumerate(self.out_names)}
            for c in range(self.n_cores)
        ]


_CACHE = {}


def _to_io(a, mm_mode):
    a = np.asarray(a, np.float32)
    if mm_mode == "bf16":
        import ml_dtypes
        return np.ascontiguousarray(a.astype(ml_dtypes.bfloat16))
    if mm_mode == "f16":
        return np.ascontiguousarray(a.astype(np.float16))
    return np.ascontiguousarray(a)


def _fp8_swizzle(a, scale, ktiles):
    """[ktiles*128, N] f32 -> fp8 device layout [128, ktiles//2, 2, N] flat."""
    import ml_dtypes
    q = np.asarray(a[:ktiles * P] * scale, ml_dtypes.float8_e4m3)
    n = q.shape[1]
    return np.ascontiguousarray(
        q.reshape(ktiles // 2, 2, P, n).transpose(2, 0, 1, 3).reshape(P, -1))


def make_in_maps(x, We1, be1, We2, be2, Wg1, bg1, Wgs, mm_mode,
                 kf8=None, gate8=None):
    if kf8 is None:
        kf8 = KF8_DEFAULT if mm_mode in ("bf16", "f16") else 0
    if gate8 is None:
        gate8 = GATE8_DEFAULT if mm_mode in ("bf16", "f16") else False
    bs = BS if mm_mode in ("bf16", "f16") else 1.0
    k0 = kf8 * P
    x0 = k0 if gate8 else 0   # bf16 x trim matches the device XOFF
    We1f = np.asarray(We1, np.float32)
    Wg1f = np.asarray(Wg1, np.float32)
    shared = {
        "We1": _to_io(We1f[:, k0:] * bs, mm_mode),
        "be1": np.ascontiguousarray(np.asarray(be1, np.float32)),
        "We2": _to_io(We2, mm_mode),
        "be2": _to_io(be2, mm_mode),
        "bg1": np.ascontiguousarray(np.asarray(bg1, np.float32)),
        "Wgs": _to_io(Wgs, mm_mode),
        "ones": _to_io(np.ones((1, P), np.float32), mm_mode),
    }
    if gate8:
        shared["Wg18"] = np.ascontiguousarray(np.stack(
            [_fp8_swizzle(Wg1f[t], WS8, KT) for t in range(T)]))
    else:
        shared["Wg1"] = _to_io(Wg1f * bs, mm_mode)
    if kf8:
        shared["We18"] = np.ascontiguousarray(np.stack(
            [_fp8_swizzle(We1f[e], WS8, kf8) for e in range(E)]))
    x = np.asarray(x, np.float32)
    in_maps = []
    for c in range(NCORES):
        xs = x[c * BC:(c + 1) * BC]
        m = {"xT": _to_io(xs.T[x0:] * bs, mm_mode), **shared}
        if kf8 or gate8:
            m["xT8"] = _fp8_swizzle(xs.T, XS8, KT if gate8 else kf8)
        in_maps.append(m)
    return in_maps


def get_runner(mm_mode="bf16", reps=1, kf8=None, gate8=None):
    if kf8 is None:
        kf8 = KF8_DEFAULT if mm_mode in ("bf16", "f16") else 0
    if gate8 is None:
        gate8 = GATE8_DEFAULT if mm_mode in ("bf16", "f16") else False
    key = (mm_mode, reps, kf8, gate8)
    if key not in _CACHE:
        nc = build_program(reps=reps, mm_mode=mm_mode, kf8=kf8, gate8=gate8)
        _CACHE[key] = SpmdRunner(nc, NCORES)
    return _CACHE[key]


MM_MODE = "f16"


def kernel(x, We1, be1, We2, be2, Wg1, bg1, Wgs):
    runner = get_runner(MM_MODE)
    in_maps = make_in_maps(x, We1, be1, We2, be2, Wg1, bg1, Wgs, MM_MODE)
    args = runner.put_inputs(in_maps)
    res = runner.results(runner.run(args))
    out = np.concatenate([r["out"] for r in res], axis=1)  # [T, B, H2]
    return np.ascontiguousarray(out.astype(np.float32))


# revision 17
# speedup vs baseline: 1.0716x; 1.0121x over previous
"""Trainium2 Bass kernel for the CGC (multi-task MoE) layer.

Reference computation (all-dense MoE, T=2 tasks, E=6 experts, EC=4 per task):
    h1 = relu(x @ We1[e] + be1[e])            [B, E, H1]
    h2 = relu(h1 @ We2[e] + be2[e])           [B, E, H2]
    g  = relu(x @ Wg1[t] + bg1[t])            [B, T, G]
    gate = softmax(g @ Wgs[t])                [B, T, EC]
    out[t, b, :] = sum_j gate[b, t, j] * h2[b, IDX[t, j], :]

Sharding: data-parallel over batch across 8 NeuronCores (B=8192 -> 1024
rows/core), weights replicated, no collectives.  The host pre-transposes each
x shard to xT[D, BC] so every matmul's contraction dim sits on SBUF
partitions with no on-device transposes:

    L1:  psum[h, b] += We1[d, h].T-block @ xT[d, b]      (lhsT=We1, rhs=xT)
    L2:  psum[b, o] += h1T[h, b].T-block @ We2[h, o]     (lhsT=h1T, rhs=We2)

Quantization plan (error budget: rel_err < 2e-2 vs the reference):
  - L1 expert matmul: first KF8=8 of 16 d-tiles as fp8e4 DoubleRow pairs,
    rest fp16.  fp8 scales x*4 / W*1024, fp16 x*64 / W*64: all products land
    in PSUM at scale 4096, descaled in the relu activation.
  - gate / L2 / logits stay fp16 (same PE rate and bytes as bf16, 8x finer
    mantissa; measured rel err 1.93e-2 vs 2.17e-2 at KF8=10 and 2.03-2.08e-2
    with an fp8 gate, whose error is heavy-tailed in the absmax metric).
fp8 DoubleRow streams ~1.13 cyc/row covering 2 k-tiles -> ~1.8x bf16 rate.
be2 is folded into the L2 PSUM accumulation with a K=1 ones-matmul.
relu+gate scaling is fused on ScalarE (gate > 0 so relu(g*x) = g*relu(x)).
"""

import contextlib

import numpy as np

import concourse.bass as bass
import concourse.mybir as mybir
import concourse.tile as tile
from concourse import bacc

# ---- problem shapes (hardcoded per contest contract) ----
B, D, H1, H2, G = 8192, 2048, 1024, 512, 512
T, SPEC, SH = 2, 2, 2
E = T * SPEC + SH          # 6
EC = SPEC + SH             # 4
NCORES = 8
BC = B // NCORES           # 1024 per-core batch
TASK_EXPERTS = [[0, 1, 4, 5], [2, 3, 4, 5]]   # IDX rows

F32 = mybir.dt.float32
F32R = mybir.dt.float32r
BF16 = mybir.dt.bfloat16
FP16 = mybir.dt.float16
FP8 = mybir.dt.float8e4
P = 128
KT = D // P                # 16
GT = G // P                # 4
HT = H1 // P               # 8
DRMODE = mybir.MatmulPerfMode.DoubleRow

# fp8 config: gate fully fp8; first KF8 of 16 L1 d-tiles as fp8 DoubleRow
# pairs.  Scales: fp8 ops at x*4 / W*1024, bf16 ops at x*64 / W*64 so every
# product lands in PSUM at scale 4096, descaled in the relu activation.
KF8_DEFAULT = 8
GATE8_DEFAULT = False
XS8, WS8, BS = 4.0, 1024.0, 64.0
PSC = 1.0 / (BS * BS)  # psum descale for the scaled accumulation

Relu = mybir.ActivationFunctionType.Relu
Exp = mybir.ActivationFunctionType.Exp
AxX = mybir.AxisListType.X


def build_program(bc=BC, bch=None, reps=1, mm_mode="bf16", psa=4,
                  psb=4, kf8=KF8_DEFAULT, gate8=GATE8_DEFAULT):
    """Build the SPMD single-core program.

    bc: per-core batch size; bch: L1/gate moving free-dim chunk; reps: body
    repetitions (for wall-clock differencing); mm_mode: f32 | f32r | bf16.
    """
    if bch is None:
        bch = min(512, bc)
    assert bc % P == 0 and bc % bch == 0
    NB, NBT = bc // bch, bc // P

    # matmul-operand dtype: walrus requires f32r matmul inputs to be produced
    # as f32r, so the whole producer chain (DRAM tensor -> DMA -> tile) is
    # declared f32r in that mode (same bytes as f32 host-side).  f16 runs at
    # the same PE rate and byte width as bf16 but with 8x finer mantissa.
    io_dt = {"bf16": BF16, "f16": FP16, "f32r": F32R, "f32": F32}[mm_mode]
    scaled = mm_mode in ("bf16", "f16")  # x/We1/Wg1 arrive pre-scaled by 64
    if not scaled:
        kf8 = 0
        gate8 = False
    psc = PSC if scaled else 1.0
    assert kf8 % 2 == 0
    KB = KT - kf8              # bf16 d-tiles (L1)
    KH = max(KB // 2, 1)       # bf16 weight tiles per half-DMA
    # bf16 x tiles resident on device: trimmed to the L1 remainder when the
    # gate path is fp8, else the full K (the bf16 gate needs all 16)
    XOFF = kf8 if gate8 else 0
    XTN = KT - XOFF
    XT8N = KT if gate8 else kf8    # fp8 x tiles resident (gate uses all 16)

    nc = bacc.Bacc("TRN2", target_bir_lowering=False, debug=False,
                   num_devices=NCORES)

    # bf16 x / We1 carry only the d-tiles not covered by fp8
    xT = nc.dram_tensor("xT", [XTN * P, bc], io_dt, kind="ExternalInput")
    We1 = nc.dram_tensor("We1", [E, KB * P, H1], io_dt, kind="ExternalInput")
    be1 = nc.dram_tensor("be1", [E, H1], F32, kind="ExternalInput")
    We2 = nc.dram_tensor("We2", [E, H1, H2], io_dt, kind="ExternalInput")
    be2 = nc.dram_tensor("be2", [E, H2], io_dt, kind="ExternalInput")
    if not gate8:
        Wg1 = nc.dram_tensor("Wg1", [T, D, G], io_dt, kind="ExternalInput")
    bg1 = nc.dram_tensor("bg1", [T, G], F32, kind="ExternalInput")
    Wgs = nc.dram_tensor("Wgs", [T, G, EC], io_dt, kind="ExternalInput")
    # K=1 all-ones lhsT used to broadcast be2 into the L2 PSUM accumulation;
    # an input tensor because only DMA can produce f32r-tagged data.
    ones_d = nc.dram_tensor("ones", [1, P], io_dt, kind="ExternalInput")
    if kf8 or gate8:
        # pre-swizzled host layout [p, kp, two, .] flattened; x carries all
        # 16 d-tiles (gate uses all of them, L1 the first kf8)
        xT8 = nc.dram_tensor("xT8", [P, XT8N * bc], FP8,
                             kind="ExternalInput")
    if kf8:
        We18 = nc.dram_tensor("We18", [E, P, kf8 * H1], FP8,
                              kind="ExternalInput")
    if gate8:
        Wg18 = nc.dram_tensor("Wg18", [T, P, KT * G], FP8,
                              kind="ExternalInput")
    out = nc.dram_tensor("out", [T, bc, H2], F32, kind="ExternalOutput")

    with tile.TileContext(nc) as tc, contextlib.ExitStack() as stack:
        ep = stack.enter_context
        pp = ep(tc.tile_pool(name="persist", bufs=1))
        # xt8 is read until the end of the expert loop; double-buffering it
        # lets the next rep's gate phase start before this rep fully drains
        xbp = ep(tc.tile_pool(name="xb", bufs=2))
        w1p = None if gate8 else ep(tc.tile_pool(name="w1", bufs=4))
        w1ep = ep(tc.tile_pool(name="w1e", bufs=2))
        w2p = ep(tc.tile_pool(name="w2", bufs=2))
        h1p = ep(tc.tile_pool(name="h1", bufs=2))
        tmpp = ep(tc.tile_pool(name="tmp", bufs=2))
        smp = ep(tc.tile_pool(name="small", bufs=2))
        psA = ep(tc.tile_pool(name="psA", bufs=psa, space="PSUM"))
        psB = ep(tc.tile_pool(name="psB", bufs=psb, space="PSUM"))

        for _rep in range(reps):
            # ---- resident x: fp8 (all d-tiles, per-kp DMAs) + bf16
            # remainder.  Gate weights land first so PE starts as soon as
            # xt8[kp=0] arrives. ----
            if gate8:
                wg18 = pp.tile([P, T * KT * G], FP8, tag="wg18")
                wg18v = wg18[:].rearrange(
                    "p (t kp two g) -> p t kp two g", t=T, two=2, g=G)
                nc.sync.dma_start(
                    wg18v[:, 0], Wg18.ap()[0].rearrange(
                        "p (kp two g) -> p kp two g", two=2, g=G))
            if kf8 or gate8:
                xt8 = xbp.tile([P, XT8N * bc], FP8, tag="xt8")
                xt8v = xt8[:].rearrange(
                    "p (kp two b) -> p kp two b", two=2, b=bc)
                xT8v = xT8.ap().rearrange(
                    "p (kp two b) -> p kp two b", two=2, b=bc)
                for kp in range(XT8N // 2):
                    nc.sync.dma_start(xt8v[:, kp], xT8v[:, kp])
                    if gate8 and kp == 1:
                        nc.sync.dma_start(
                            wg18v[:, 1], Wg18.ap()[1].rearrange(
                                "p (kp two g) -> p kp two g", two=2, g=G))

            pairs = [(t, gg) for t in range(T) for gg in range(GT)]
            w1vs = {}

            def load_gate_w1(t, gg):
                w1 = w1p.tile([P, KT * P], io_dt, tag="w1",
                              name=f"w1g_{t}_{gg}")
                w1v = w1[:].rearrange("p (n h) -> p n h", h=P)
                nc.sync.dma_start(
                    w1v,
                    Wg1.ap()[t].rearrange("(n p) g -> p n g", p=P)
                    [:, :, gg * P:(gg + 1) * P])
                w1vs[(t, gg)] = w1v

            if not gate8:
                load_gate_w1(*pairs[0])

            # bf16 x remainder (the L1 d-tiles >= kf8), per-d DMAs.
            # The first NA tiles are double-buffered so the next rep's gate
            # phase can start right behind this rep's final L2 instead of
            # stalling on the write-after-read hazard against this rep's L1.
            NA = min(6, XTN)
            xta = xbp.tile([P, NA * bc], io_dt, tag="xta")
            xtav = xta[:].rearrange("p (n b) -> p n b", b=bc)
            if XTN > NA:
                xtb = pp.tile([P, (XTN - NA) * bc], io_dt, tag="xtb")
                xtbv = xtb[:].rearrange("p (n b) -> p n b", b=bc)

            def xtile(d):
                return xtav[:, d] if d < NA else xtbv[:, d - NA]

            xTv = xT.ap().rearrange("(n p) b -> n p b", p=P)
            w1_at = {} if gate8 else {0: pairs[1], 1: pairs[2], 4: pairs[3]}
            for d in range(XTN):
                nc.sync.dma_start(xtile(d), xTv[d])
                if d in w1_at:
                    load_gate_w1(*w1_at[d])
            # L1's bf16 remainder lives at this offset
            XL1 = kf8 - XOFF

            ones = pp.tile([1, P], io_dt, tag="ones")
            nc.sync.dma_start(ones[:], ones_d[:])

            # all experts' biases in one DMA each
            be1a = pp.tile([P, E * HT], F32, tag="be1a")
            be1av = be1a[:].rearrange("p (e n) -> p e n", n=HT)
            nc.sync.dma_start(
                be1av, be1.ap().rearrange("e (n p) -> p e n", p=P))
            be2a = pp.tile([1, E * H2], io_dt, tag="be2a")
            be2av = be2a[:].rearrange("q (e o) -> q e o", o=H2)
            nc.sync.dma_start(be2av, be2.ap()[None, :, :])

            # ---- gate phase ----
            # Both tasks' gate-hidden activations live in one h1-pool slot
            # [P, T, GT, bc].  Chains run >=4-wide with the contraction loop
            # OUTER so early matmuls track the x tiles arriving from HBM.
            bgta = smp.tile([P, T * GT], F32, tag="bg")
            bgtav = bgta[:].rearrange("p (t n) -> p t n", n=GT)
            nc.sync.dma_start(
                bgtav, bg1.ap().rearrange("t (n p) -> p t n", p=P))
            bgts = [bgtav[:, t, :] for t in range(T)]
            wgsa = smp.tile([P, T * GT * EC], io_dt, tag="wgs")
            wgsav = wgsa[:].rearrange("p (t n e) -> p t n e", n=GT, e=EC)
            nc.sync.dma_start(
                wgsav, Wgs.ap().rearrange("t (n p) e -> p t n e", p=P))
            wgsvs = [wgsav[:, t] for t in range(T)]

            gt_all = h1p.tile([P, T * GT * bc], io_dt, tag="h1T")
            gtv = gt_all[:].rearrange("p (t n b) -> p t n b", n=GT, b=bc)

            gatesvs = []
            for t in range(T):
                gates_t = pp.tile([P, NBT * EC], F32, tag=f"gates{t}")
                gatesvs.append(gates_t[:].rearrange("p (n e) -> p n e",
                                                    e=EC))

            def emit_logits(t):
                # tiny 4-col logit matmuls share one psum bank (column
                # slices) so they pipeline; softmax runs on DVE/ScalarE
                # underneath the remaining PE stream.
                gatesv = gatesvs[t]
                for bt0 in range(0, NBT, 2):
                    bts = [bt0, bt0 + 1]
                    pszs = [psB.tile([P, EC], F32, tag="psB",
                                     name=f"psz_{t}_{bt}")
                            for bt in bts]
                    for gg in range(GT):
                        for i in range(2):
                            nc.tensor.matmul(
                                pszs[i][:],
                                gtv[:, t, gg,
                                    bts[i] * P:(bts[i] + 1) * P],
                                wgsvs[t][:, gg, :],
                                start=(gg == 0), stop=(gg == GT - 1))
                    for i, bt in enumerate(bts):
                        psz = pszs[i]
                        mx = smp.tile([P, 1], F32, tag="mx")
                        nc.vector.reduce_max(mx[:], psz[:], axis=AxX)
                        sh = smp.tile([P, EC], F32, tag="sh")
                        nc.vector.tensor_scalar_sub(sh[:], psz[:], mx[:])
                        ex = smp.tile([P, EC], F32, tag="ex")
                        ssum = smp.tile([P, 1], F32, tag="ss")
                        nc.scalar.activation(ex[:], sh[:], Exp,
                                             accum_out=ssum[:])
                        rec = smp.tile([P, 1], F32, tag="rc")
                        nc.vector.reciprocal(rec[:], ssum[:])
                        nc.vector.tensor_scalar_mul(gatesv[:, bt, :],
                                                    ex[:], rec[:])

            # groups of (t, g) pairs, 4 chains wide on psA ONLY: the psB
            # ring is still draining the previous rep's L2, and the PE queue
            # is in-order, so a gate chain on psB would stall the whole
            # stream at the rep boundary.  t=0's hiddens exist after group
            # 1 and t=1's after group 3, so each task's logits+softmax
            # issue early and overlap the remaining matmul stream.
            groups = [pairs[0:2], pairs[2:4], pairs[4:6], pairs[6:8]]
            for gi, grp in enumerate(groups):
                if not gate8:
                    for (t, gg) in grp:
                        if (t, gg) not in w1vs:
                            load_gate_w1(t, gg)
                chains = [(t, gg, cb) for (t, gg) in grp
                          for cb in range(NB)]
                pss = {}
                for i, c in enumerate(chains):
                    pss[c] = psA.tile(
                        [P, bch], F32, tag="psA",
                        name=f"psg_{c[0]}_{c[1]}_{c[2]}")
                if gate8:
                    for kp in range(KT // 2):
                        for (t, gg, cb) in chains:
                            nc.tensor.matmul(
                                pss[(t, gg, cb)][:],
                                wg18v[:, t, kp, :, gg * P:(gg + 1) * P],
                                xt8v[:, kp, :, cb * bch:(cb + 1) * bch],
                                start=(kp == 0), stop=(kp == KT // 2 - 1),
                                perf_mode=DRMODE, skip_group_check=True)
                else:
                    for d in range(KT):
                        for (t, gg, cb) in chains:
                            nc.tensor.matmul(
                                pss[(t, gg, cb)][:],
                                w1vs[(t, gg)][:, d, :],
                                xtile(d)[:, cb * bch:(cb + 1) * bch],
                                start=(d == 0), stop=(d == KT - 1))
                for (t, gg, cb) in chains:
                    nc.scalar.activation(
                        gtv[:, t, gg, cb * bch:(cb + 1) * bch],
                        pss[(t, gg, cb)][:], Relu, scale=psc,
                        bias=bgts[t][:, gg:gg + 1])
                if gi == 1:
                    emit_logits(0)
                if gi == 3:
                    emit_logits(1)

            gates = gatesvs

            # ---- output accumulators ----
            accs = []
            for t in range(T):
                acc = pp.tile([P, NBT * H2], F32, tag=f"acc{t}")
                accs.append(acc[:].rearrange("p (n o) -> p n o", o=H2))

            # ---- expert loop ----
            n_seen = [0, 0]
            for e in range(E):
                w2t = w2p.tile([P, HT * H2], io_dt, tag="w2")
                w2v = w2t[:].rearrange("p (n o) -> p n o", o=H2)
                nc.sync.dma_start(
                    w2v, We2.ap()[e].rearrange("(n p) o -> p n o", p=P))
                w2vs = [w2v[:, ht, :] for ht in range(HT)]
                be2t = be2av[:, e, :]
                be1t = be1av[:, e, :]
                if kf8:
                    w18 = w2p.tile([P, kf8 * H1], FP8, tag="w18")
                    nc.sync.dma_start(w18[:], We18.ap()[e])
                    w18v = w18[:].rearrange(
                        "p (kp two h) -> p kp two h", two=2, h=H1)

                # be2 broadcast to [P, H2] once per expert (one K=1
                # ones-matmul + copy), DVE-added into each L2 psum below
                psb2 = psB.tile([P, H2], F32, tag="psB")
                nc.tensor.matmul(psb2[:], ones[:], be2t,
                                 start=True, stop=True)
                be2b = smp.tile([P, H2], F32, tag="be2b")
                nc.scalar.copy(be2b[:], psb2[:])

                h1 = h1p.tile([P, HT * bc], io_dt, tag="h1T")
                h1v = h1[:].rearrange("p (n b) -> p n b", b=bc)

                # L1: h1T[h, b] = relu(sum_d We1[d, h]^T x[d, b] + be1)
                # bf16 expert weights arrive in two half-K DMAs; the NB
                # b-chunk chains interleave per-d so consecutive matmuls
                # share the same stationary weights and chain boundaries
                # overlap (psA double-buffers 2 ht groups)
                w1evs = []
                for half in range(2):
                    w1e = w1ep.tile([P, KH * H1], io_dt, tag="w1e")
                    w1ev = w1e[:].rearrange("p (n h) -> p n h", h=H1)
                    nc.sync.dma_start(
                        w1ev,
                        We1.ap()[e].rearrange("(n p) h -> p n h", p=P)
                        [:, half * KH:(half + 1) * KH, :])
                    w1evs.append(w1ev)
                for ht in range(HT):
                    pss1 = [psA.tile([P, bch], F32, tag="psA",
                                     name=f"ps1_{e}_{ht}_{cb}")
                            for cb in range(NB)]
                    # chain: first bf16 remainder tile opens the bank
                    # full-width, then the fp8 DoubleRow pairs, then the
                    # rest of the bf16 tiles
                    for cb in range(NB):
                        nc.tensor.matmul(
                            pss1[cb][:],
                            w1evs[0][:, 0, ht * P:(ht + 1) * P],
                            xtile(XL1)[:, cb * bch:(cb + 1) * bch],
                            start=True, stop=False)
                    for kp in range(kf8 // 2):
                        for cb in range(NB):
                            nc.tensor.matmul(
                                pss1[cb][:],
                                w18v[:, kp, :, ht * P:(ht + 1) * P],
                                xt8v[:, kp, :, cb * bch:(cb + 1) * bch],
                                start=False, stop=False,
                                perf_mode=DRMODE,
                                skip_group_check=True)
                    for d in range(1, KB):
                        for cb in range(NB):
                            nc.tensor.matmul(
                                pss1[cb][:],
                                w1evs[d // KH][:, d % KH,
                                               ht * P:(ht + 1) * P],
                                xtile(XL1 + d)[:, cb * bch:(cb + 1) * bch],
                                start=False, stop=(d == KB - 1))
                    for cb in range(NB):
                        nc.scalar.activation(
                            h1v[:, ht, cb * bch:(cb + 1) * bch],
                            pss1[cb][:], Relu, scale=psc,
                            bias=be1t[:, ht:ht + 1])

                # L2 + gated accumulation; btile pairs interleave so
                # consecutive matmuls share the moving w2 operand and chain
                # boundaries overlap (psB double-buffers)
                for t in range(T):
                    if e in TASK_EXPERTS[t]:
                        n_seen[t] += 1
                for bt0 in range(0, NBT, 2):
                    bts = [bt0, bt0 + 1]
                    pss2 = [psB.tile([P, H2], F32, tag="psB",
                                     name=f"ps2_{e}_{bt}")
                            for bt in bts]
                    for ht in range(HT):
                        for i, bt in enumerate(bts):
                            nc.tensor.matmul(
                                pss2[i][:],
                                h1v[:, ht, bt * P:(bt + 1) * P],
                                w2vs[ht],
                                start=(ht == 0), stop=(ht == HT - 1))
                    for i, bt in enumerate(bts):
                        ps2 = pss2[i]
                        nc.vector.tensor_add(ps2[:], ps2[:], be2b[:])
                        for t in range(T):
                            if e not in TASK_EXPERTS[t]:
                                continue
                            j = TASK_EXPERTS[t].index(e)
                            gate_ap = gates[t][:, bt, j:j + 1]
                            if n_seen[t] == 1:
                                nc.scalar.activation(
                                    accs[t][:, bt, :], ps2[:], Relu,
                                    scale=gate_ap)
                            else:
                                tmp = tmpp.tile([P, H2], F32, tag="tmp")
                                nc.scalar.activation(tmp[:], ps2[:],
                                                     Relu, scale=gate_ap)
                                nc.vector.tensor_add(accs[t][:, bt, :],
                                                     accs[t][:, bt, :],
                                                     tmp[:])
                            if n_seen[t] == EC:
                                nc.sync.dma_start(
                                    out.ap()[t].rearrange(
                                        "(n p) o -> p n o", p=P)
                                    [:, bt, :],
                                    accs[t][:, bt, :])

    nc.compile()
    return nc


# ---------------------------------------------------------------------------
# host-side SPMD execution (mirrors bass_utils.run_bass_kernel_spmd's axon
# path, but keeps the jitted callable so repeat calls don't recompile)
# ---------------------------------------------------------------------------
class SpmdRunner:
    def __init__(self, nc, n_cores):
        import jax
        from jax.sharding import Mesh, PartitionSpec
        from jax.experimental.shard_map import shard_map
        from concourse.bass2jax import (_bass_exec_p, install_neuronx_cc_hook,
                                        partition_id_tensor)
        install_neuronx_cc_hook()
        self.jax = jax
        self.nc = nc
        self.n_cores = n_cores
        partition_name = (nc.partition_id_tensor.name
                          if nc.partition_id_tensor else None)
        in_names, out_names, out_avals, zero_outs = [], [], [], []
        for alloc in nc.m.functions[0].allocations:
            if not isinstance(alloc, mybir.MemoryLocationSet):
                continue
            name = alloc.memorylocations[0].name
            if alloc.kind == "ExternalInput":
                if name != partition_name:
                    in_names.append(name)
            elif alloc.kind == "ExternalOutput":
                out_names.append(name)
                shape = tuple(alloc.tensor_shape)
                dtype = mybir.dt.np(alloc.dtype)
                out_avals.append(jax.core.ShapedArray(shape, dtype))
                zero_outs.append(np.zeros(shape, dtype))
        all_in_names = list(in_names) + list(out_names)
        if partition_name is not None:
            all_in_names.append(partition_name)

        def _body(*args):
            operands = list(args)
            if partition_name is not None:
                operands.append(partition_id_tensor())
            outs = _bass_exec_p.bind(
                *operands,
                out_avals=tuple(out_avals),
                in_names=tuple(all_in_names),
                out_names=tuple(out_names),
                lowering_input_output_aliases=(),
                sim_require_finite=True,
                sim_require_nnan=True,
                nc=nc,
            )
            return tuple(outs)

        devices = jax.devices()[:n_cores]
        assert len(devices) == n_cores
        self.mesh = Mesh(np.asarray(devices), ("core",))
        n_args = len(in_names) + len(out_names)
        self.fn = jax.jit(
            shard_map(_body, mesh=self.mesh,
                      in_specs=(PartitionSpec("core"),) * n_args,
                      out_specs=(PartitionSpec("core"),) * len(out_names),
                      check_rep=False),
            keep_unused=True,
        )
        self.in_names = in_names
        self.out_names = out_names
        self.out_avals = out_avals
        self.zero_outs = zero_outs
        self.PartitionSpec = PartitionSpec

    def put_inputs(self, in_maps):
        jax = self.jax
        concat_in = [
            np.concatenate([np.asarray(m[name]) for m in in_maps], axis=0)
            for name in self.in_names
        ]
        concat_zeros = [
            np.zeros((self.n_cores * z.shape[0], *z.shape[1:]), z.dtype)
            for z in self.zero_outs
        ]
        sh = jax.sharding.NamedSharding(self.mesh, self.PartitionSpec("core"))
        return [jax.device_put(a, sh) for a in concat_in + concat_zeros]

    def run(self, args):
        out = self.fn(*args)
        self.jax.block_until_ready(out)
        return out

    def results(self, out_arrs):
        return [
            {name: np.asarray(out_arrs[i]).reshape(
                self.n_cores, *self.out_avals[i].shape)[c]
             for i, name in enumerate(self.out_names)}
            for c in range(self.n_cores)
        ]


_CACHE = {}


def _to_io(a, mm_mode):
    a = np.asarray(a, np.float32)
    if mm_mode == "bf16":
        import ml_dtypes
        return np.ascontiguousarray(a.astype(ml_dtypes.bfloat16))
    if mm_mode == "f16":
        return np.ascontiguousarray(a.astype(np.float16))
    return np.ascontiguousarray(a)


def _fp8_swizzle(a, scale, ktiles):
    """[ktiles*128, N] f32 -> fp8 device layout [128, ktiles//2, 2, N] flat."""
    import ml_dtypes
    q = np.asarray(a[:ktiles * P] * scale, ml_dtypes.float8_e4m3)
    n = q.shape[1]
    return np.ascontiguousarray(
        q.reshape(ktiles // 2, 2, P, n).transpose(2, 0, 1, 3).reshape(P, -1))


def make_in_maps(x, We1, be1, We2, be2, Wg1, bg1, Wgs, mm_mode,
                 kf8=None, gate8=None):
    if kf8 is None:
        kf8 = KF8_DEFAULT if mm_mode in ("bf16", "f16") else 0
    if gate8 is None:
        gate8 = GATE8_DEFAULT if mm_mode in ("bf16", "f16") else False
    bs = BS if mm_mode in ("bf16", "f16") else 1.0
    k0 = kf8 * P
    x0 = k0 if gate8 else 0   # bf16 x trim matches the device XOFF
    We1f = np.asarray(We1, np.float32)
    Wg1f = np.asarray(Wg1, np.float32)
    shared = {
        "We1": _to_io(We1f[:, k0:] * bs, mm_mode),
        "be1": np.ascontiguousarray(np.asarray(be1, np.float32)),
        "We2": _to_io(We2, mm_mode),
        "be2": _to_io(be2, mm_mode),
        "bg1": np.ascontiguousarray(np.asarray(bg1, np.float32)),
        "Wgs": _to_io(Wgs, mm_mode),
        "ones": _to_io(np.ones((1, P), np.float32), mm_mode),
    }
    if gate8:
        shared["Wg18"] = np.ascontiguousarray(np.stack(
            [_fp8_swizzle(Wg1f[t], WS8, KT) for t in range(T)]))
    else:
        shared["Wg1"] = _to_io(Wg1f * bs, mm_mode)
    if kf8:
        shared["We18"] = np.ascontiguousarray(np.stack(
            [_fp8_swizzle(We1f[e], WS8, kf8) for e in range(E)]))
    x = np.asarray(x, np.float32)
    in_maps = []
    for c in range(NCORES):
        xs = x[c * BC:(c + 1) * BC]
        m = {"xT": _to_io(xs.T[x0:] * bs, mm_mode), **shared}
        if kf8 or gate8:
            m["xT8"] = _fp8_swizzle(xs.T, XS8, KT if gate8 else kf8)
        in_maps.append(m)
    return in_maps


def get_runner(mm_mode="bf16", reps=1, kf8=None, gate8=None):
    if kf8 is None:
        kf8 = KF8_DEFAULT if mm_mode in ("bf16", "f16") else 0
    if gate8 is None:
        gate8 = GATE8_DEFAULT if mm_mode in ("bf16", "f16") else False
    key = (mm_mode, reps, kf8, gate8)
    if key not in _CACHE:
        nc = build_program(reps=reps, mm_mode=mm_mode, kf8=kf8, gate8=gate8)
        _CACHE[key] = SpmdRunner(nc, NCORES)
    return _CACHE[key]


MM_MODE = "f16"


def kernel(x, We1, be1, We2, be2, Wg1, bg1, Wgs):
    runner = get_runner(MM_MODE)
    in_maps = make_in_maps(x, We1, be1, We2, be2, Wg1, bg1, Wgs, MM_MODE)
    args = runner.put_inputs(in_maps)
    res = runner.results(runner.run(args))
    out = np.concatenate([r["out"] for r in res], axis=1)  # [T, B, H2]
    return np.ascontiguousarray(out.astype(np.float32))


# revision 20
# speedup vs baseline: 1.0892x; 1.0164x over previous
"""Trainium2 Bass kernel for the CGC (multi-task MoE) layer.

Reference computation (all-dense MoE, T=2 tasks, E=6 experts, EC=4 per task):
    h1 = relu(x @ We1[e] + be1[e])            [B, E, H1]
    h2 = relu(h1 @ We2[e] + be2[e])           [B, E, H2]
    g  = relu(x @ Wg1[t] + bg1[t])            [B, T, G]
    gate = softmax(g @ Wgs[t])                [B, T, EC]
    out[t, b, :] = sum_j gate[b, t, j] * h2[b, IDX[t, j], :]

Sharding: data-parallel over batch across 8 NeuronCores (B=8192 -> 1024
rows/core), weights replicated, no collectives.  The host pre-transposes each
x shard to xT[D, BC] so every matmul's contraction dim sits on SBUF
partitions with no on-device transposes:

    L1:  psum[h, b] += We1[d, h].T-block @ xT[d, b]      (lhsT=We1, rhs=xT)
    L2:  psum[b, o] += h1T[h, b].T-block @ We2[h, o]     (lhsT=h1T, rhs=We2)

Quantization plan (error budget: rel_err < 2e-2 vs the reference):
  - L1 expert matmul: first KF8=8 of 16 d-tiles as fp8e4 DoubleRow pairs,
    rest fp16.  fp8 scales x*4 / W*1024, fp16 x*64 / W*64: all products land
    in PSUM at scale 4096, descaled in the relu activation.
  - gate / L2 / logits stay fp16 (same PE rate and bytes as bf16, 8x finer
    mantissa; measured rel err 1.93e-2 vs 2.17e-2 at KF8=10 and 2.03-2.08e-2
    with an fp8 gate, whose error is heavy-tailed in the absmax metric).
fp8 DoubleRow streams ~1.13 cyc/row covering 2 k-tiles -> ~1.8x bf16 rate.
be2 is folded into the L2 PSUM accumulation with a K=1 ones-matmul.
relu+gate scaling is fused on ScalarE (gate > 0 so relu(g*x) = g*relu(x)).
"""

import contextlib

import numpy as np

import concourse.bass as bass
import concourse.mybir as mybir
import concourse.tile as tile
from concourse import bacc

# ---- problem shapes (hardcoded per contest contract) ----
B, D, H1, H2, G = 8192, 2048, 1024, 512, 512
T, SPEC, SH = 2, 2, 2
E = T * SPEC + SH          # 6
EC = SPEC + SH             # 4
NCORES = 8
BC = B // NCORES           # 1024 per-core batch
TASK_EXPERTS = [[0, 1, 4, 5], [2, 3, 4, 5]]   # IDX rows

F32 = mybir.dt.float32
F32R = mybir.dt.float32r
BF16 = mybir.dt.bfloat16
FP16 = mybir.dt.float16
FP8 = mybir.dt.float8e4
P = 128
KT = D // P                # 16
GT = G // P                # 4
HT = H1 // P               # 8
DRMODE = mybir.MatmulPerfMode.DoubleRow

# fp8 config: gate fully fp8; first KF8 of 16 L1 d-tiles as fp8 DoubleRow
# pairs.  Scales: fp8 ops at x*4 / W*1024, bf16 ops at x*64 / W*64 so every
# product lands in PSUM at scale 4096, descaled in the relu activation.
KF8_DEFAULT = 8
GATE8_DEFAULT = False
XS8, WS8, BS = 4.0, 1024.0, 64.0
PSC = 1.0 / (BS * BS)  # psum descale for the scaled accumulation

Relu = mybir.ActivationFunctionType.Relu
Exp = mybir.ActivationFunctionType.Exp
AxX = mybir.AxisListType.X


def build_program(bc=BC, bch=None, reps=1, mm_mode="bf16", psa=4,
                  psb=4, kf8=KF8_DEFAULT, gate8=GATE8_DEFAULT,
                  phase="all"):
    """Build the SPMD single-core program.

    bc: per-core batch size; bch: L1/gate moving free-dim chunk; reps: body
    repetitions (for wall-clock differencing); mm_mode: f32 | f32r | bf16.
    """
    if bch is None:
        bch = min(512, bc)
    assert bc % P == 0 and bc % bch == 0
    NB, NBT = bc // bch, bc // P

    # matmul-operand dtype: walrus requires f32r matmul inputs to be produced
    # as f32r, so the whole producer chain (DRAM tensor -> DMA -> tile) is
    # declared f32r in that mode (same bytes as f32 host-side).  f16 runs at
    # the same PE rate and byte width as bf16 but with 8x finer mantissa.
    io_dt = {"bf16": BF16, "f16": FP16, "f32r": F32R, "f32": F32}[mm_mode]
    scaled = mm_mode in ("bf16", "f16")  # x/We1/Wg1 arrive pre-scaled by 64
    if not scaled:
        kf8 = 0
        gate8 = False
    psc = PSC if scaled else 1.0
    assert kf8 % 2 == 0
    KB = KT - kf8              # bf16 d-tiles (L1)
    KH = max(KB // 2, 1)       # bf16 weight tiles per half-DMA
    # bf16 x tiles resident on device: trimmed to the L1 remainder when the
    # gate path is fp8, else the full K (the bf16 gate needs all 16)
    XOFF = kf8 if gate8 else 0
    XTN = KT - XOFF
    XT8N = KT if gate8 else kf8    # fp8 x tiles resident (gate uses all 16)

    nc = bacc.Bacc("TRN2", target_bir_lowering=False, debug=False,
                   num_devices=NCORES)

    # bf16 x / We1 carry only the d-tiles not covered by fp8
    xT = nc.dram_tensor("xT", [XTN * P, bc], io_dt, kind="ExternalInput")
    We1 = nc.dram_tensor("We1", [E, KB * P, H1], io_dt, kind="ExternalInput")
    be1 = nc.dram_tensor("be1", [E, H1], F32, kind="ExternalInput")
    We2 = nc.dram_tensor("We2", [E, H1, H2], io_dt, kind="ExternalInput")
    be2 = nc.dram_tensor("be2", [E, H2], io_dt, kind="ExternalInput")
    if not gate8:
        # host pre-layout: entry t*GT+gg = Wg1[t][:, gg*128:(gg+1)*128]
        # rearranged to [p, d, h] so each gate-weight DMA is contiguous
        Wg1 = nc.dram_tensor("Wg1", [T * GT, P, KT * P], io_dt,
                             kind="ExternalInput")
    bg1 = nc.dram_tensor("bg1", [T, G], F32, kind="ExternalInput")
    Wgs = nc.dram_tensor("Wgs", [T, G, EC], io_dt, kind="ExternalInput")
    # K=1 all-ones lhsT used to broadcast be2 into the L2 PSUM accumulation;
    # an input tensor because only DMA can produce f32r-tagged data.
    ones_d = nc.dram_tensor("ones", [1, P], io_dt, kind="ExternalInput")
    if kf8 or gate8:
        # pre-swizzled host layout [p, kp, two, .] flattened; x carries all
        # 16 d-tiles (gate uses all of them, L1 the first kf8)
        xT8 = nc.dram_tensor("xT8", [P, XT8N * bc], FP8,
                             kind="ExternalInput")
    if kf8:
        We18 = nc.dram_tensor("We18", [E, P, kf8 * H1], FP8,
                              kind="ExternalInput")
    if gate8:
        Wg18 = nc.dram_tensor("Wg18", [T, P, KT * G], FP8,
                              kind="ExternalInput")
    out = nc.dram_tensor("out", [T, bc, H2], F32, kind="ExternalOutput")

    with tile.TileContext(nc) as tc, contextlib.ExitStack() as stack:
        ep = stack.enter_context
        pp = ep(tc.tile_pool(name="persist", bufs=1))
        # xt8 is read until the end of the expert loop; double-buffering it
        # lets the next rep's gate phase start before this rep fully drains
        xbp = ep(tc.tile_pool(name="xb", bufs=2))
        w1p = None if gate8 else ep(tc.tile_pool(name="w1", bufs=4))
        w1ep = ep(tc.tile_pool(name="w1e", bufs=2))
        w2p = ep(tc.tile_pool(name="w2", bufs=2))
        h1p = ep(tc.tile_pool(name="h1", bufs=2))
        tmpp = ep(tc.tile_pool(name="tmp", bufs=2))
        smp = ep(tc.tile_pool(name="small", bufs=2))
        psA = ep(tc.tile_pool(name="psA", bufs=psa, space="PSUM"))
        psB = ep(tc.tile_pool(name="psB", bufs=psb, space="PSUM"))

        for _rep in range(reps):
            # ---- resident x: fp8 (all d-tiles, per-kp DMAs) + bf16
            # remainder.  Gate weights land first so PE starts as soon as
            # xt8[kp=0] arrives. ----
            if gate8:
                wg18 = pp.tile([P, T * KT * G], FP8, tag="wg18")
                wg18v = wg18[:].rearrange(
                    "p (t kp two g) -> p t kp two g", t=T, two=2, g=G)
                nc.sync.dma_start(
                    wg18v[:, 0], Wg18.ap()[0].rearrange(
                        "p (kp two g) -> p kp two g", two=2, g=G))
            if kf8 or gate8:
                xt8 = xbp.tile([P, XT8N * bc], FP8, tag="xt8")
                xt8v = xt8[:].rearrange(
                    "p (kp two b) -> p kp two b", two=2, b=bc)
                xT8v = xT8.ap().rearrange(
                    "p (kp two b) -> p kp two b", two=2, b=bc)
                for kp in range(XT8N // 2):
                    nc.sync.dma_start(xt8v[:, kp], xT8v[:, kp])
                    if gate8 and kp == 1:
                        nc.sync.dma_start(
                            wg18v[:, 1], Wg18.ap()[1].rearrange(
                                "p (kp two g) -> p kp two g", two=2, g=G))

            pairs = [(t, gg) for t in range(T) for gg in range(GT)]
            w1vs = {}

            def load_gate_w1(t, gg):
                w1 = w1p.tile([P, KT * P], io_dt, tag="w1",
                              name=f"w1g_{t}_{gg}")
                nc.sync.dma_start(w1[:], Wg1.ap()[t * GT + gg])
                w1vs[(t, gg)] = w1[:].rearrange("p (n h) -> p n h", h=P)

            if not gate8:
                load_gate_w1(*pairs[0])

            # bf16 x remainder (the L1 d-tiles >= kf8), per-d DMAs.
            # The first NA tiles are double-buffered so the next rep's gate
            # phase can start right behind this rep's final L2 instead of
            # stalling on the write-after-read hazard against this rep's L1.
            NA = min(6, XTN)
            xta = xbp.tile([P, NA * bc], io_dt, tag="xta")
            xtav = xta[:].rearrange("p (n b) -> p n b", b=bc)
            if XTN > NA:
                xtb = pp.tile([P, (XTN - NA) * bc], io_dt, tag="xtb")
                xtbv = xtb[:].rearrange("p (n b) -> p n b", b=bc)

            def xtile(d):
                return xtav[:, d] if d < NA else xtbv[:, d - NA]

            xTv = xT.ap().rearrange("(n p) b -> n p b", p=P)
            w1_at = {} if gate8 else {0: pairs[1], 1: pairs[2], 4: pairs[3]}
            for d in range(XTN):
                nc.sync.dma_start(xtile(d), xTv[d])
                if d in w1_at:
                    load_gate_w1(*w1_at[d])
            # L1's bf16 remainder lives at this offset
            XL1 = kf8 - XOFF

            ones = pp.tile([1, P], io_dt, tag="ones")
            nc.sync.dma_start(ones[:], ones_d[:])

            # all experts' biases in one DMA each
            be1a = pp.tile([P, E * HT], F32, tag="be1a")
            be1av = be1a[:].rearrange("p (e n) -> p e n", n=HT)
            nc.sync.dma_start(
                be1av, be1.ap().rearrange("e (n p) -> p e n", p=P))
            be2a = pp.tile([1, E * H2], io_dt, tag="be2a")
            be2av = be2a[:].rearrange("q (e o) -> q e o", o=H2)
            nc.sync.dma_start(be2av, be2.ap()[None, :, :])

            # ---- gate phase ----
            # Both tasks' gate-hidden activations live in one h1-pool slot
            # [P, T, GT, bc].  Chains run >=4-wide with the contraction loop
            # OUTER so early matmuls track the x tiles arriving from HBM.
            bgta = smp.tile([P, T * GT], F32, tag="bg")
            bgtav = bgta[:].rearrange("p (t n) -> p t n", n=GT)
            nc.sync.dma_start(
                bgtav, bg1.ap().rearrange("t (n p) -> p t n", p=P))
            bgts = [bgtav[:, t, :] for t in range(T)]
            wgsa = smp.tile([P, T * GT * EC], io_dt, tag="wgs")
            wgsav = wgsa[:].rearrange("p (t n e) -> p t n e", n=GT, e=EC)
            nc.sync.dma_start(
                wgsav, Wgs.ap().rearrange("t (n p) e -> p t n e", p=P))
            wgsvs = [wgsav[:, t] for t in range(T)]

            gt_all = h1p.tile([P, T * GT * bc], io_dt, tag="h1T")
            gtv = gt_all[:].rearrange("p (t n b) -> p t n b", n=GT, b=bc)

            gatesvs = []
            for t in range(T):
                gates_t = pp.tile([P, NBT * EC], F32, tag=f"gates{t}")
                if phase == "experts":
                    # timing ablation only: fill gates with arbitrary data
                    nc.scalar.copy(gates_t[:], be1a[:, :NBT * EC])
                gatesvs.append(gates_t[:].rearrange("p (n e) -> p n e",
                                                    e=EC))

            def emit_logits(t):
                # tiny 4-col logit matmuls share one psum bank (column
                # slices) so they pipeline; softmax runs on DVE/ScalarE
                # underneath the remaining PE stream.
                gatesv = gatesvs[t]
                for bt0 in range(0, NBT, 2):
                    bts = [bt0, bt0 + 1]
                    pszs = [psB.tile([P, EC], F32, tag="psB",
                                     name=f"psz_{t}_{bt}")
                            for bt in bts]
                    for gg in range(GT):
                        for i in range(2):
                            nc.tensor.matmul(
                                pszs[i][:],
                                gtv[:, t, gg,
                                    bts[i] * P:(bts[i] + 1) * P],
                                wgsvs[t][:, gg, :],
                                start=(gg == 0), stop=(gg == GT - 1))
                    for i, bt in enumerate(bts):
                        psz = pszs[i]
                        mx = smp.tile([P, 1], F32, tag="mx")
                        nc.vector.reduce_max(mx[:], psz[:], axis=AxX)
                        sh = smp.tile([P, EC], F32, tag="sh")
                        nc.vector.tensor_scalar_sub(sh[:], psz[:], mx[:])
                        ex = smp.tile([P, EC], F32, tag="ex")
                        ssum = smp.tile([P, 1], F32, tag="ss")
                        nc.scalar.activation(ex[:], sh[:], Exp,
                                             accum_out=ssum[:])
                        rec = smp.tile([P, 1], F32, tag="rc")
                        nc.vector.reciprocal(rec[:], ssum[:])
                        nc.vector.tensor_scalar_mul(gatesv[:, bt, :],
                                                    ex[:], rec[:])

            # groups of (t, g) pairs, 4 chains wide on psA ONLY: the psB
            # ring is still draining the previous rep's L2, and the PE queue
            # is in-order, so a gate chain on psB would stall the whole
            # stream at the rep boundary.  t=0's hiddens exist after group
            # 1 and t=1's after group 3, so each task's logits+softmax
            # issue early and overlap the remaining matmul stream.
            groups = [pairs[0:2], pairs[2:4], pairs[4:6], pairs[6:8]]
            if phase == "experts":
                groups = []
            for gi, grp in enumerate(groups):
                if not gate8:
                    for (t, gg) in grp:
                        if (t, gg) not in w1vs:
                            load_gate_w1(t, gg)
                chains = [(t, gg, cb) for (t, gg) in grp
                          for cb in range(NB)]
                pss = {}
                for i, c in enumerate(chains):
                    pss[c] = psA.tile(
                        [P, bch], F32, tag="psA",
                        name=f"psg_{c[0]}_{c[1]}_{c[2]}")
                if gate8:
                    for kp in range(KT // 2):
                        for (t, gg, cb) in chains:
                            nc.tensor.matmul(
                                pss[(t, gg, cb)][:],
                                wg18v[:, t, kp, :, gg * P:(gg + 1) * P],
                                xt8v[:, kp, :, cb * bch:(cb + 1) * bch],
                                start=(kp == 0), stop=(kp == KT // 2 - 1),
                                perf_mode=DRMODE, skip_group_check=True)
                else:
                    for d in range(KT):
                        for (t, gg, cb) in chains:
                            nc.tensor.matmul(
                                pss[(t, gg, cb)][:],
                                w1vs[(t, gg)][:, d, :],
                                xtile(d)[:, cb * bch:(cb + 1) * bch],
                                start=(d == 0), stop=(d == KT - 1))
                for (t, gg, cb) in chains:
                    nc.scalar.activation(
                        gtv[:, t, gg, cb * bch:(cb + 1) * bch],
                        pss[(t, gg, cb)][:], Relu, scale=psc,
                        bias=bgts[t][:, gg:gg + 1])
                if gi == 1:
                    emit_logits(0)
                if gi == 3:
                    emit_logits(1)

            gates = gatesvs

            # ---- output accumulators ----
            accs = []
            for t in range(T):
                acc = pp.tile([P, NBT * H2], F32, tag=f"acc{t}")
                accs.append(acc[:].rearrange("p (n o) -> p n o", o=H2))

            # ---- expert loop ----
            n_seen = [0, 0]
            for e in range(E if phase != "gate" else 0):
                w2t = w2p.tile([P, HT * H2], io_dt, tag="w2")
                w2v = w2t[:].rearrange("p (n o) -> p n o", o=H2)
                nc.sync.dma_start(
                    w2v, We2.ap()[e].rearrange("(n p) o -> p n o", p=P))
                w2vs = [w2v[:, ht, :] for ht in range(HT)]
                be2t = be2av[:, e, :]
                be1t = be1av[:, e, :]
                if kf8:
                    w18 = w2p.tile([P, kf8 * H1], FP8, tag="w18")
                    nc.sync.dma_start(w18[:], We18.ap()[e])
                    w18v = w18[:].rearrange(
                        "p (kp two h) -> p kp two h", two=2, h=H1)

                # be2 broadcast to [P, H2] once per expert (one K=1
                # ones-matmul + copy), DVE-added into each L2 psum below
                psb2 = psB.tile([P, H2], F32, tag="psB")
                nc.tensor.matmul(psb2[:], ones[:], be2t,
                                 start=True, stop=True)
                be2b = smp.tile([P, H2], F32, tag="be2b")
                nc.scalar.copy(be2b[:], psb2[:])

                h1 = h1p.tile([P, HT * bc], io_dt, tag="h1T")
                h1v = h1[:].rearrange("p (n b) -> p n b", b=bc)

                # L1: h1T[h, b] = relu(sum_d We1[d, h]^T x[d, b] + be1)
                # bf16 expert weights arrive in two half-K DMAs; the NB
                # b-chunk chains interleave per-d so consecutive matmuls
                # share the same stationary weights and chain boundaries
                # overlap (psA double-buffers 2 ht groups)
                w1evs = []
                for half in range(2):
                    w1e = w1ep.tile([P, KH * H1], io_dt, tag="w1e")
                    w1ev = w1e[:].rearrange("p (n h) -> p n h", h=H1)
                    nc.sync.dma_start(
                        w1ev,
                        We1.ap()[e].rearrange("(n p) h -> p n h", p=P)
                        [:, half * KH:(half + 1) * KH, :])
                    w1evs.append(w1ev)
                for ht in range(HT):
                    pss1 = [psA.tile([P, bch], F32, tag="psA",
                                     name=f"ps1_{e}_{ht}_{cb}")
                            for cb in range(NB)]
                    # chain: first bf16 remainder tile opens the bank
                    # full-width, then the fp8 DoubleRow pairs, then the
                    # rest of the bf16 tiles
                    for cb in range(NB):
                        nc.tensor.matmul(
                            pss1[cb][:],
                            w1evs[0][:, 0, ht * P:(ht + 1) * P],
                            xtile(XL1)[:, cb * bch:(cb + 1) * bch],
                            start=True, stop=False)
                    for kp in range(kf8 // 2):
                        for cb in range(NB):
                            nc.tensor.matmul(
                                pss1[cb][:],
                                w18v[:, kp, :, ht * P:(ht + 1) * P],
                                xt8v[:, kp, :, cb * bch:(cb + 1) * bch],
                                start=False, stop=False,
                                perf_mode=DRMODE,
                                skip_group_check=True)
                    for d in range(1, KB):
                        for cb in range(NB):
                            nc.tensor.matmul(
                                pss1[cb][:],
                                w1evs[d // KH][:, d % KH,
                                               ht * P:(ht + 1) * P],
                                xtile(XL1 + d)[:, cb * bch:(cb + 1) * bch],
                                start=False, stop=(d == KB - 1))
                    for cb in range(NB):
                        nc.scalar.activation(
                            h1v[:, ht, cb * bch:(cb + 1) * bch],
                            pss1[cb][:], Relu, scale=psc,
                            bias=be1t[:, ht:ht + 1])

                # L2 + gated accumulation; btile pairs interleave so
                # consecutive matmuls share the moving w2 operand and chain
                # boundaries overlap (psB double-buffers)
                for t in range(T):
                    if e in TASK_EXPERTS[t]:
                        n_seen[t] += 1
                for bt0 in range(0, NBT, 2):
                    bts = [bt0, bt0 + 1]
                    pss2 = [psB.tile([P, H2], F32, tag="psB",
                                     name=f"ps2_{e}_{bt}")
                            for bt in bts]
                    for ht in range(HT):
                        for i, bt in enumerate(bts):
                            nc.tensor.matmul(
                                pss2[i][:],
                                h1v[:, ht, bt * P:(bt + 1) * P],
                                w2vs[ht],
                                start=(ht == 0), stop=(ht == HT - 1))
                    for i, bt in enumerate(bts):
                        ps2 = pss2[i]
                        nc.vector.tensor_add(ps2[:], ps2[:], be2b[:])
                        for t in range(T):
                            if e not in TASK_EXPERTS[t]:
                                continue
                            j = TASK_EXPERTS[t].index(e)
                            gate_ap = gates[t][:, bt, j:j + 1]
                            if n_seen[t] == 1:
                                nc.scalar.activation(
                                    accs[t][:, bt, :], ps2[:], Relu,
                                    scale=gate_ap)
                            else:
                                tmp = tmpp.tile([P, H2], F32, tag="tmp")
                                nc.scalar.activation(tmp[:], ps2[:],
                                                     Relu, scale=gate_ap)
                                nc.vector.tensor_add(accs[t][:, bt, :],
                                                     accs[t][:, bt, :],
                                                     tmp[:])
                            if n_seen[t] == EC:
                                nc.sync.dma_start(
                                    out.ap()[t].rearrange(
                                        "(n p) o -> p n o", p=P)
                                    [:, bt, :],
                                    accs[t][:, bt, :])

    nc.compile()
    return nc


# ---------------------------------------------------------------------------
# host-side SPMD execution (mirrors bass_utils.run_bass_kernel_spmd's axon
# path, but keeps the jitted callable so repeat calls don't recompile)
# ---------------------------------------------------------------------------
class SpmdRunner:
    def __init__(self, nc, n_cores):
        import jax
        from jax.sharding import Mesh, PartitionSpec
        from jax.experimental.shard_map import shard_map
        from concourse.bass2jax import (_bass_exec_p, install_neuronx_cc_hook,
                                        partition_id_tensor)
        install_neuronx_cc_hook()
        self.jax = jax
        self.nc = nc
        self.n_cores = n_cores
        partition_name = (nc.partition_id_tensor.name
                          if nc.partition_id_tensor else None)
        in_names, out_names, out_avals, zero_outs = [], [], [], []
        for alloc in nc.m.functions[0].allocations:
            if not isinstance(alloc, mybir.MemoryLocationSet):
                continue
            name = alloc.memorylocations[0].name
            if alloc.kind == "ExternalInput":
                if name != partition_name:
                    in_names.append(name)
            elif alloc.kind == "ExternalOutput":
                out_names.append(name)
                shape = tuple(alloc.tensor_shape)
                dtype = mybir.dt.np(alloc.dtype)
                out_avals.append(jax.core.ShapedArray(shape, dtype))
                zero_outs.append(np.zeros(shape, dtype))
        all_in_names = list(in_names) + list(out_names)
        if partition_name is not None:
            all_in_names.append(partition_name)

        def _body(*args):
            operands = list(args)
            if partition_name is not None:
                operands.append(partition_id_tensor())
            outs = _bass_exec_p.bind(
                *operands,
                out_avals=tuple(out_avals),
                in_names=tuple(all_in_names),
                out_names=tuple(out_names),
                lowering_input_output_aliases=(),
                sim_require_finite=True,
                sim_require_nnan=True,
                nc=nc,
            )
            return tuple(outs)

        devices = jax.devices()[:n_cores]
        assert len(devices) == n_cores
        self.mesh = Mesh(np.asarray(devices), ("core",))
        n_args = len(in_names) + len(out_names)
        self.fn = jax.jit(
            shard_map(_body, mesh=self.mesh,
                      in_specs=(PartitionSpec("core"),) * n_args,
                      out_specs=(PartitionSpec("core"),) * len(out_names),
                      check_rep=False),
            keep_unused=True,
        )
        self.in_names = in_names
        self.out_names = out_names
        self.out_avals = out_avals
        self.zero_outs = zero_outs
        self.PartitionSpec = PartitionSpec

    def put_inputs(self, in_maps):
        jax = self.jax
        concat_in = [
            np.concatenate([np.asarray(m[name]) for m in in_maps], axis=0)
            for name in self.in_names
        ]
        concat_zeros = [
            np.zeros((self.n_cores * z.shape[0], *z.shape[1:]), z.dtype)
            for z in self.zero_outs
        ]
        sh = jax.sharding.NamedSharding(self.mesh, self.PartitionSpec("core"))
        return [jax.device_put(a, sh) for a in concat_in + concat_zeros]

    def run(self, args):
        out = self.fn(*args)
        self.jax.block_until_ready(out)
        return out

    def results(self, out_arrs):
        return [
            {name: np.asarray(out_arrs[i]).reshape(
                self.n_cores, *self.out_avals[i].shape)[c]
             for i, name in enumerate(self.out_names)}
            for c in range(self.n_cores)
        ]


_CACHE = {}


def _to_io(a, mm_mode):
    a = np.asarray(a, np.float32)
    if mm_mode == "bf16":
        import ml_dtypes
        return np.ascontiguousarray(a.astype(ml_dtypes.bfloat16))
    if mm_mode == "f16":
        return np.ascontiguousarray(a.astype(np.float16))
    return np.ascontiguousarray(a)


def _fp8_swizzle(a, scale, ktiles):
    """[ktiles*128, N] f32 -> fp8 device layout [128, ktiles//2, 2, N] flat."""
    import ml_dtypes
    q = np.asarray(a[:ktiles * P] * scale, ml_dtypes.float8_e4m3)
    n = q.shape[1]
    return np.ascontiguousarray(
        q.reshape(ktiles // 2, 2, P, n).transpose(2, 0, 1, 3).reshape(P, -1))


def make_in_maps(x, We1, be1, We2, be2, Wg1, bg1, Wgs, mm_mode,
                 kf8=None, gate8=None):
    if kf8 is None:
        kf8 = KF8_DEFAULT if mm_mode in ("bf16", "f16") else 0
    if gate8 is None:
        gate8 = GATE8_DEFAULT if mm_mode in ("bf16", "f16") else False
    bs = BS if mm_mode in ("bf16", "f16") else 1.0
    k0 = kf8 * P
    x0 = k0 if gate8 else 0   # bf16 x trim matches the device XOFF
    We1f = np.asarray(We1, np.float32)
    Wg1f = np.asarray(Wg1, np.float32)
    shared = {
        "We1": _to_io(We1f[:, k0:] * bs, mm_mode),
        "be1": np.ascontiguousarray(np.asarray(be1, np.float32)),
        "We2": _to_io(We2, mm_mode),
        "be2": _to_io(be2, mm_mode),
        "bg1": np.ascontiguousarray(np.asarray(bg1, np.float32)),
        "Wgs": _to_io(Wgs, mm_mode),
        "ones": _to_io(np.ones((1, P), np.float32), mm_mode),
    }
    if gate8:
        shared["Wg18"] = np.ascontiguousarray(np.stack(
            [_fp8_swizzle(Wg1f[t], WS8, KT) for t in range(T)]))
    else:
        shared["Wg1"] = _to_io(np.stack(
            [Wg1f[t][:, gg * P:(gg + 1) * P]
             .reshape(KT, P, P).transpose(1, 0, 2).reshape(P, KT * P) * bs
             for t in range(T) for gg in range(GT)]), mm_mode)
    if kf8:
        shared["We18"] = np.ascontiguousarray(np.stack(
            [_fp8_swizzle(We1f[e], WS8, kf8) for e in range(E)]))
    x = np.asarray(x, np.float32)
    in_maps = []
    for c in range(NCORES):
        xs = x[c * BC:(c + 1) * BC]
        m = {"xT": _to_io(xs.T[x0:] * bs, mm_mode), **shared}
        if kf8 or gate8:
            m["xT8"] = _fp8_swizzle(xs.T, XS8, KT if gate8 else kf8)
        in_maps.append(m)
    return in_maps


def get_runner(mm_mode="bf16", reps=1, kf8=None, gate8=None,
               phase="all"):
    if kf8 is None:
        kf8 = KF8_DEFAULT if mm_mode in ("bf16", "f16") else 0
    if gate8 is None:
        gate8 = GATE8_DEFAULT if mm_mode in ("bf16", "f16") else False
    key = (mm_mode, reps, kf8, gate8, phase)
    if key not in _CACHE:
        nc = build_program(reps=reps, mm_mode=mm_mode, kf8=kf8, gate8=gate8,
                           phase=phase)
        _CACHE[key] = SpmdRunner(nc, NCORES)
    return _CACHE[key]


MM_MODE = "f16"


def kernel(x, We1, be1, We2, be2, Wg1, bg1, Wgs):
    runner = get_runner(MM_MODE)
    in_maps = make_in_maps(x, We1, be1, We2, be2, Wg1, bg1, Wgs, MM_MODE)
    args = runner.put_inputs(in_maps)
    res = runner.results(runner.run(args))
    out = np.concatenate([r["out"] for r in res], axis=1)  # [T, B, H2]
    return np.ascontiguousarray(out.astype(np.float32))
